# revision 5
# baseline (speedup 1.0000x reference)
"""Trainium2 Bass kernel for nn_CrispToFuzzyConv (hypergraph message passing).

Math (see reference):
  Xe   = segment_sum(X[vertex], edges, E)                 # round 1
  Xv   = segment_sum(concat([X[vertex], Xe[edges]]), vertex, N)
       = concat([deg * X, Xv2]),  Xv2 = segment_sum(Xe[edges], vertex, N)
  center = Xv @ w_b + b_b
  HL = center - (|Xv| @ w_a + b_a)
  HR = center + (|Xv| @ w_c + b_c)

Distribution over 8 NeuronCores:
  - round 1 sharded by edge owner (edge e -> core e // 6250): local
    dma_gather from replicated X + dma_scatter_add into per-core Xe shard
  - AllGather of Xe shards (6400-row padded shards -> 51200-row table)
  - round 2 sharded by vertex owner: gather from Xe_full + scatter into
    per-core Xv2 shard
  - dense stage per core: deg-scaling, PE transposes, 13 matmuls/tile
    (bias folded in as a K=1 ones x bias matmul)

Key hardware constraints baked in (established empirically):
  - dma_gather/dma_scatter_add indices are int16 -> gather tables are
    chunked to <= 32768 rows; <= 1024 indices per call (129-descriptor
    ring limit at 2048)
  - duplicate scatter rows WITHIN one call race (lost updates) -> tokens
    are dealt round-robin over tiles so each call's rows are unique;
    pad slots scatter garbage to junk rows (never read)
  - consecutive scatter calls are serialized by Tile (WAW) -> exact
"""

import os
import numpy as np

# ---------------------------------------------------------------- constants
N = 100000
E = 50000
NNZ = 300000
F = 128
NC = 8

NODE_SH = N // NC            # 12500
EDGE_SH = E // NC            # 6250
NODE_SH_P = 12544            # 98 * 128 (padded; rows >= 12500 are junk)
EDGE_SH_P = 6400             # 50 * 128 (padded; rows >= 6250 are junk)
XE_FULL = NC * EDGE_SH_P     # 51200

T = 1024                     # tokens per gather/scatter call
A_CHUNKS = 4                 # X gather table chunks (25000 rows, int16-safe)
A_CHUNK_ROWS = N // A_CHUNKS
A_TPC = 10                   # stage-A tiles per (core, chunk) segment
C_CHUNKS = 2                 # Xe_full gather chunks (25600 rows)
C_CHUNK_ROWS = XE_FULL // C_CHUNKS
C_TPC = 20                   # stage-C tiles per (core, chunk) segment
A_TILES = A_CHUNKS * A_TPC   # 40
C_TILES = C_CHUNKS * C_TPC   # 40
ND_TILES = NODE_SH_P // 128  # 98

MM_DT = os.environ.get("BASS_GNN_MM_DT", "f32")  # "f32" | "bf16"
STAGES = os.environ.get("BASS_GNN_STAGES", "ZABCD")

_STATE = {}


# ---------------------------------------------------------------- host side
def _wrap16(idx):
    """[n_tiles, T] int -> [n_tiles, 128, T//16] int16 (idx i at partition
    i%16, col i//16; replicated across the 8 groups of 16 partitions)."""
    n_tiles = idx.shape[0]
    t = idx.reshape(n_tiles, T // 16, 16).transpose(0, 2, 1).astype(np.int16)
    return np.ascontiguousarray(np.tile(t, (1, 8, 1)))


def _deal(g, s, n_tiles, junk_base, junk_n):
    """Pack a segment's tokens into n_tiles gather/scatter idx tiles.

    g, s: gather idx / scatter row per token, with s sorted ascending.
    Round-robin dealing (token p -> tile p % n_tiles, slot p // n_tiles)
    guarantees unique scatter rows per tile when every row's multiplicity
    <= n_tiles. Pad slots gather row 0 and scatter to junk rows.
    """
    n = len(g)
    if n > n_tiles * T or (n and np.bincount(s).max() > n_tiles):
        return None
    ga = np.zeros((n_tiles, T), np.int64)
    sa = np.empty((n_tiles, T), np.int64)
    sa[:] = junk_base + (np.arange(T) % junk_n)[None, :]
    p = np.arange(n)
    ga[p % n_tiles, p // n_tiles] = g
    sa[p % n_tiles, p // n_tiles] = s
    return ga, sa


def _route(vertex, edges):
    """Build per-core idx tensors for both gather/scatter stages.
    Returns None if the (astronomically unlikely) static capacities are
    exceeded -> caller falls back to numpy."""
    ga_all, sa_all, gc_all, sc_all = [], [], [], []
    owner_a = edges // EDGE_SH
    chunk_a = vertex // A_CHUNK_ROWS
    xe_row = EDGE_SH_P * (edges // EDGE_SH) + (edges % EDGE_SH)
    owner_c = vertex // NODE_SH
    chunk_c = xe_row // C_CHUNK_ROWS
    for m in range(NC):
        ga_m = np.zeros((A_TILES, T), np.int64)
        sa_m = np.empty((A_TILES, T), np.int64)
        sa_m[:] = EDGE_SH + (np.arange(T) % (EDGE_SH_P - EDGE_SH))[None, :]
        for c in range(A_CHUNKS):
            sel = np.nonzero((owner_a == m) & (chunk_a == c))[0]
            s = edges[sel] - EDGE_SH * m
            order = np.argsort(s, kind="stable")
            d = _deal(vertex[sel][order] - A_CHUNK_ROWS * c, s[order], A_TPC,
                      EDGE_SH, EDGE_SH_P - EDGE_SH)
            if d is None:
                return None
            ga_m[c * A_TPC:(c + 1) * A_TPC] = d[0]
            sa_m[c * A_TPC:(c + 1) * A_TPC] = d[1]
        gc_m = np.zeros((C_TILES, T), np.int64)
        sc_m = np.empty((C_TILES, T), np.int64)
        sc_m[:] = NODE_SH + (np.arange(T) % (NODE_SH_P - NODE_SH))[None, :]
        for c in range(C_CHUNKS):
            sel = np.nonzero((owner_c == m) & (chunk_c == c))[0]
            s = vertex[sel] - NODE_SH * m
            order = np.argsort(s, kind="stable")
            d = _deal(xe_row[sel][order] - C_CHUNK_ROWS * c, s[order], C_TPC,
                      NODE_SH, NODE_SH_P - NODE_SH)
            if d is None:
                return None
            gc_m[c * C_TPC:(c + 1) * C_TPC] = d[0]
            sc_m[c * C_TPC:(c + 1) * C_TPC] = d[1]
        ga_all.append(_wrap16(ga_m))
        sa_all.append(_wrap16(sa_m))
        gc_all.append(_wrap16(gc_m))
        sc_all.append(_wrap16(sc_m))
    return ga_all, sa_all, gc_all, sc_all


def _numpy_fallback(X, vertex, edges, w_b, w_a, w_c, b_b, b_a, b_c):
    Xe = np.zeros((E, F), np.float32)
    np.add.at(Xe, edges, X[vertex])
    Xv2 = np.zeros((N, F), np.float32)
    np.add.at(Xv2, vertex, Xe[edges])
    deg = np.bincount(vertex, minlength=N).astype(np.float32)[:, None]
    Xv = np.concatenate([deg * X, Xv2], axis=1)
    center = Xv @ w_b + b_b
    aXv = np.abs(Xv)
    return (center.astype(np.float32),
            (center - (aXv @ w_a + b_a)).astype(np.float32),
            (center + (aXv @ w_c + b_c)).astype(np.float32))


# ------------------------------------------------------------- bass program
def _build_program():
    from concourse import bacc, tile
    import concourse.mybir as mybir

    f32 = mybir.dt.float32
    mmdt = mybir.dt.bfloat16 if MM_DT == "bf16" else f32
    i16 = mybir.dt.int16

    nc = bacc.Bacc(None, target_bir_lowering=False, debug=False,
                   num_devices=NC, num_swdge_queues=4)

    xfull = nc.dram_tensor("xfull", [N, F], f32, kind="ExternalInput")
    xshard = nc.dram_tensor("xshard", [NODE_SH, F], f32, kind="ExternalInput")
    ga = nc.dram_tensor("ga", [A_TILES, 128, T // 16], i16, kind="ExternalInput")
    sa = nc.dram_tensor("sa", [A_TILES, 128, T // 16], i16, kind="ExternalInput")
    gc = nc.dram_tensor("gc", [C_TILES, 128, T // 16], i16, kind="ExternalInput")
    sc = nc.dram_tensor("sc", [C_TILES, 128, T // 16], i16, kind="ExternalInput")
    deg = nc.dram_tensor("deg", [ND_TILES, 128, 1], f32, kind="ExternalInput")
    wts_d = {nm: nc.dram_tensor(nm, [F, F], mmdt, kind="ExternalInput")
             for nm in ("wb1", "wb2", "wa1n", "wa2n", "wc1", "wc2")}
    bias_d = {nm: nc.dram_tensor(nm, [1, F], mmdt, kind="ExternalInput")
              for nm in ("bias_c", "bias_l", "bias_r")}
    outs = {nm: nc.dram_tensor(nm, [NODE_SH, F], f32, kind="ExternalOutput")
            for nm in ("center", "hl", "hr")}

    xe_a = nc.dram_tensor("xe_a", [EDGE_SH_P, F], f32)
    xe_b = nc.dram_tensor("xe_b", [EDGE_SH_P, F], f32)
    xe_sum = nc.dram_tensor("xe_sum", [EDGE_SH_P, F], f32)
    xe_full = nc.dram_tensor("xe_full", [XE_FULL, F], f32)
    xv2_a = nc.dram_tensor("xv2_a", [NODE_SH_P, F], f32)
    xv2_b = nc.dram_tensor("xv2_b", [NODE_SH_P, F], f32)

    eye_d = nc.inline_tensor(np.eye(128, dtype=np.float32), name="eye128")

    with tile.TileContext(nc) as tc:
        with (
            tc.tile_pool(name="cpool", bufs=1) as cpool,
            tc.tile_pool(name="ipool", bufs=4) as ipool,
            tc.tile_pool(name="dpool", bufs=4) as dpool,
            tc.tile_pool(name="spool", bufs=3) as spool,
            tc.tile_pool(name="tpool", bufs=2) as tpool,
            tc.tile_pool(name="opool", bufs=2) as opool,
            tc.tile_pool(name="ps_tr", bufs=2, space="PSUM") as ps_tr,
            tc.tile_pool(name="ps_mm", bufs=1, space="PSUM") as ps_mm,
        ):
            # constants
            zero = cpool.tile([128, 2048], f32)
            nc.vector.memset(zero[:], 0.0)
            ident = cpool.tile([128, 128], f32)
            nc.sync.dma_start(ident[:], eye_d[:])
            ones = cpool.tile([1, F], mmdt)
            nc.vector.memset(ones[:], 1.0)
            wts = {}
            for nm, d in wts_d.items():
                wtile = cpool.tile([F, F], mmdt, tag=nm)
                nc.sync.dma_start(wtile[:], d[:])
                wts[nm] = wtile
            biases = {}
            for nm, d in bias_d.items():
                btile = cpool.tile([1, F], mmdt, tag=nm)
                nc.sync.dma_start(btile[:], d[:])
                biases[nm] = btile

            # zero the DRAM accumulators
            flats = [t.ap().rearrange("(p a) f -> p (a f)", p=128)
                     for t in (xe_a, xe_b, xv2_a, xv2_b)]
            for flat in flats if "Z" in STAGES else ():
                w = flat.shape[1]
                for off in range(0, w, 2048):
                    sz = min(2048, w - off)
                    nc.sync.dma_start(flat[:, off:off + sz], zero[:, :sz])

            # stage A: X[vertex] scatter-added by edge
            for ti in range(A_TILES if "A" in STAGES else 0):
                c = ti // A_TPC
                gt = ipool.tile([128, T // 16], i16, tag="gt")
                st = ipool.tile([128, T // 16], i16, tag="st")
                nc.sync.dma_start(gt[:], ga[ti])
                nc.sync.dma_start(st[:], sa[ti])
                dat = dpool.tile([128, T // 128, F], f32, tag="dat")
                nc.gpsimd.dma_gather(
                    dat[:], xfull[c * A_CHUNK_ROWS:(c + 1) * A_CHUNK_ROWS, :],
                    gt[:], T, T, F, queue_num=(0 if ti % 2 == 0 else 2))
                nc.gpsimd.dma_scatter_add(
                    xe_a[:] if ti % 2 == 0 else xe_b[:], dat[:], st[:], T, T, F,
                    queue_num=(1 if ti % 2 == 0 else 3))

            # stage A.5: xe_sum = xe_a + xe_b
            if "A" in STAGES:
                xa_f = xe_a.ap().rearrange("(p a) f -> p (a f)", p=128)
                xb_f = xe_b.ap().rearrange("(p a) f -> p (a f)", p=128)
                xs_f = xe_sum.ap().rearrange("(p a) f -> p (a f)", p=128)
                w_tot = xa_f.shape[1]
                for off in range(0, w_tot, 1600):
                    sz = min(1600, w_tot - off)
                    ta = spool.tile([128, 1600], f32, tag="sum_a")
                    tb = spool.tile([128, 1600], f32, tag="sum_b")
                    nc.sync.dma_start(ta[:, :sz], xa_f[:, off:off + sz])
                    nc.sync.dma_start(tb[:, :sz], xb_f[:, off:off + sz])
                    nc.vector.tensor_add(ta[:, :sz], ta[:, :sz], tb[:, :sz])
                    nc.sync.dma_start(xs_f[:, off:off + sz], ta[:, :sz])

            # stage A.5: xe_sum = xe_a + xe_b
            if "A" in STAGES:
                xa_f = xe_a.ap().rearrange("(p a) f -> p (a f)", p=128)
                xb_f = xe_b.ap().rearrange("(p a) f -> p (a f)", p=128)
                xs_f = xe_sum.ap().rearrange("(p a) f -> p (a f)", p=128)
                w_tot = xa_f.shape[1]
                for off in range(0, w_tot, 1600):
                    sz = min(1600, w_tot - off)
                    ta = spool.tile([128, 1600], f32, tag="sum_a")
                    tb = spool.tile([128, 1600], f32, tag="sum_b")
                    nc.sync.dma_start(ta[:, :sz], xa_f[:, off:off + sz])
                    nc.sync.dma_start(tb[:, :sz], xb_f[:, off:off + sz])
                    nc.vector.tensor_add(ta[:, :sz], ta[:, :sz], tb[:, :sz])
                    nc.sync.dma_start(xs_f[:, off:off + sz], ta[:, :sz])

            # stage B: all-gather the Xe shards
            if "B" in STAGES:
                nc.gpsimd.collective_compute(
                    "AllGather", mybir.AluOpType.bypass,
                    replica_groups=[list(range(NC))],
                    ins=[xe_sum.ap().opt()],
                    outs=[xe_full.ap().opt()],
                )

            # stage C: Xe[edges] scatter-added by vertex
            c_src = xfull if os.environ.get("BASS_GNN_CSRC") == "xfull" else xe_full
            for ti in range(C_TILES if "C" in STAGES else 0):
                c = ti // C_TPC
                gt = ipool.tile([128, T // 16], i16, tag="gt")
                st = ipool.tile([128, T // 16], i16, tag="st")
                nc.sync.dma_start(gt[:], gc[ti])
                nc.sync.dma_start(st[:], sc[ti])
                dat = dpool.tile([128, T // 128, F], f32, tag="dat")
                nc.gpsimd.dma_gather(
                    dat[:], c_src[c * C_CHUNK_ROWS:(c + 1) * C_CHUNK_ROWS, :],
                    gt[:], T, T, F, queue_num=(0 if ti % 2 == 0 else 2))
                nc.gpsimd.dma_scatter_add(
                    xv2_a[:] if ti % 2 == 0 else xv2_b[:], dat[:], st[:], T, T, F,
                    queue_num=(1 if ti % 2 == 0 else 3))

            # stage D: dense head
            Abs = mybir.ActivationFunctionType.Abs
            Copy = mybir.ActivationFunctionType.Copy
            for nt in range(ND_TILES if "D" in STAGES else 0):
                rows = min(128, NODE_SH - nt * 128)
                r0 = nt * 128
                xt = spool.tile([128, F], f32, tag="xt")
                nc.sync.dma_start(xt[:rows, :], xshard[r0:r0 + rows, :])
                dg = spool.tile([128, 1], f32, tag="dg")
                nc.sync.dma_start(dg[:], deg[nt])
                xv2t = spool.tile([128, F], f32, tag="xv2t")
                nc.sync.dma_start(xv2t[:], xv2_a[r0:r0 + 128, :])
                xv2t_b = spool.tile([128, F], f32, tag="xv2t_b")
                nc.sync.dma_start(xv2t_b[:], xv2_b[r0:r0 + 128, :])
                nc.vector.tensor_add(xv2t[:], xv2t[:], xv2t_b[:])

                h1 = spool.tile([128, F], f32, tag="h1")
                nc.scalar.activation(h1[:], xt[:], Copy, scale=dg[:, 0:1])

                h1T_ps = ps_tr.tile([128, F], f32, tag="h1T_ps")
                nc.tensor.transpose(h1T_ps[:], h1[:], ident[:])
                h2T_ps = ps_tr.tile([128, F], f32, tag="h2T_ps")
                nc.tensor.transpose(h2T_ps[:], xv2t[:], ident[:])

                h1T = tpool.tile([128, F], mmdt, tag="h1T")
                nc.vector.tensor_copy(h1T[:], h1T_ps[:])
                h2T = tpool.tile([128, F], mmdt, tag="h2T")
                nc.vector.tensor_copy(h2T[:], h2T_ps[:])
                a1T = tpool.tile([128, F], mmdt, tag="a1T")
                nc.scalar.activation(a1T[:], h1T_ps[:], Abs)
                a2T = tpool.tile([128, F], mmdt, tag="a2T")
                nc.scalar.activation(a2T[:], h2T_ps[:], Abs)

                groups = (
                    ("c_ps", "bias_c", (("h1T", "wb1"), ("h2T", "wb2"))),
                    ("l_ps", "bias_l", (("h1T", "wb1"), ("h2T", "wb2"),
                                        ("a1T", "wa1n"), ("a2T", "wa2n"))),
                    ("r_ps", "bias_r", (("h1T", "wb1"), ("h2T", "wb2"),
                                        ("a1T", "wc1"), ("a2T", "wc2"))),
                )
                lhs = {"h1T": h1T, "h2T": h2T, "a1T": a1T, "a2T": a2T}
                ps_out = {}
                for psname, bias, terms in groups:
                    ps = ps_mm.tile([128, F], f32, tag=psname)
                    nc.tensor.matmul(ps[:], ones[:], biases[bias][:],
                                     start=True, stop=False)
                    for i, (ln, wn) in enumerate(terms):
                        nc.tensor.matmul(ps[:], lhs[ln][:], wts[wn][:],
                                         start=False, stop=(i == len(terms) - 1))
                    ps_out[psname] = ps
                for psname, oname in (("c_ps", "center"), ("l_ps", "hl"),
                                      ("r_ps", "hr")):
                    ot = opool.tile([128, F], f32, tag=f"o_{oname}")
                    nc.vector.tensor_copy(ot[:], ps_out[psname][:])
                    nc.sync.dma_start(outs[oname][r0:r0 + rows, :], ot[:rows, :])

    nc.compile()
    return nc


# ------------------------------------------------------------------- driver
def kernel(X, vertex, edges, X0, n_edges, w_b, w_a, w_c, b_b, b_a, b_c):
    from concourse.bass_utils import run_bass_kernel_spmd
    import ml_dtypes

    X = np.ascontiguousarray(np.asarray(X, dtype=np.float32))
    vertex = np.asarray(vertex).astype(np.int64)
    edges = np.asarray(edges).astype(np.int64)
    w_b = np.asarray(w_b, dtype=np.float32)
    w_a = np.asarray(w_a, dtype=np.float32)
    w_c = np.asarray(w_c, dtype=np.float32)
    b_b = np.asarray(b_b, dtype=np.float32).reshape(1, F)
    b_a = np.asarray(b_a, dtype=np.float32).reshape(1, F)
    b_c = np.asarray(b_c, dtype=np.float32).reshape(1, F)

    routed = _route(vertex, edges)
    if routed is None:
        return _numpy_fallback(X, vertex, edges, w_b, w_a, w_c, b_b, b_a, b_c)
    ga_all, sa_all, gc_all, sc_all = routed

    if "nc" not in _STATE:
        _STATE["nc"] = _build_program()
    nc = _STATE["nc"]

    npmm = ml_dtypes.bfloat16 if MM_DT == "bf16" else np.float32
    deg_full = np.bincount(vertex, minlength=N).astype(np.float32)
    wmats = {
        "wb1": w_b[:F], "wb2": w_b[F:],
        "wa1n": -w_a[:F], "wa2n": -w_a[F:],
        "wc1": w_c[:F], "wc2": w_c[F:],
    }
    bmats = {"bias_c": b_b, "bias_l": b_b - b_a, "bias_r": b_b + b_c}

    in_maps = []
    for m in range(NC):
        dshard = np.zeros(NODE_SH_P, np.float32)
        dshard[:NODE_SH] = deg_full[m * NODE_SH:(m + 1) * NODE_SH]
        im = {
            "xfull": X,
            "xshard": np.ascontiguousarray(X[m * NODE_SH:(m + 1) * NODE_SH]),
            "ga": ga_all[m], "sa": sa_all[m],
            "gc": gc_all[m], "sc": sc_all[m],
            "deg": dshard.reshape(ND_TILES, 128, 1),
        }
        for nm, w in wmats.items():
            im[nm] = np.ascontiguousarray(w.astype(npmm))
        for nm, b in bmats.items():
            im[nm] = np.ascontiguousarray(b.astype(npmm))
        in_maps.append(im)

    res = run_bass_kernel_spmd(nc, in_maps, list(range(NC)))
    center = np.concatenate([res.results[m]["center"] for m in range(NC)])
    hl = np.concatenate([res.results[m]["hl"] for m in range(NC)])
    hr = np.concatenate([res.results[m]["hr"] for m in range(NC)])
    return center, hl, hr


# revision 7
# speedup vs baseline: 1.0038x; 1.0038x over previous
"""Trainium2 Bass kernel for nn_CrispToFuzzyConv (hypergraph message passing).

Math (see reference):
  Xe   = segment_sum(X[vertex], edges, E)                 # round 1
  Xv   = segment_sum(concat([X[vertex], Xe[edges]]), vertex, N)
       = concat([deg * X, Xv2]),  Xv2 = segment_sum(Xe[edges], vertex, N)
  center = Xv @ w_b + b_b
  HL = center - (|Xv| @ w_a + b_a)
  HR = center + (|Xv| @ w_c + b_c)

Distribution over 8 NeuronCores:
  - round 1 sharded by edge owner: dma_gather rows from replicated X,
    dma_scatter_add into 4 round-robin per-core Xe accumulators (4
    independent WAW chains on 4 SWDGE queues), summed into the padded
    Xe shard
  - 2 AllGather collectives (one per edge region) -> two 25600-row Xe
    tables; region-0 stage-C work starts while region-1 is still landing
  - round 2 sharded by vertex owner: gather from the Xe tables + scatter
    into 4 Xv2 accumulators
  - dense stage per core: deg-scaling (ACT per-partition scale), PE
    transposes, 13 matmuls/tile with bias folded in as K=1 ones x bias

Key hardware constraints baked in (established empirically):
  - dma_gather/dma_scatter_add indices are int16 -> gather tables are
    chunked to <= 32768 rows; <= 1024 indices per call (the SWDGE ring
    rejects 1280+)
  - duplicate scatter rows WITHIN one call race (lost updates) -> tokens
    are dealt round-robin over a segment's tiles so each call's rows are
    unique; pad slots gather row 0 and scatter garbage to junk rows
  - consecutive scatter calls to one tensor serialize (Tile WAW) and
    accumulate exactly -> 4 alternating accumulators give 4 parallel
    chains
  - collective in/out tensors must be Internal, addr_space Local (Shared
    breaks dma_gather reading the output)
  - num_swdge_queues=4 parallelizes Q7 descriptor generation ~3x
"""

import os
import numpy as np

# ---------------------------------------------------------------- constants
N = 100000
E = 50000
NNZ = 300000
F = 128
NC = 8

NODE_SH = N // NC            # 12500
EDGE_SH = E // NC            # 6250
NODE_SH_P = 12544            # 98 * 128 (rows >= 12500 are junk)
EDGE_REG = EDGE_SH // 2      # 3125 real edges per region
EDGE_REG_P = 3200            # 25 * 128 (rows >= 3125 are junk)
EDGE_SH_P = 2 * EDGE_REG_P   # 6400 (regions stacked)
XE_TBL = NC * EDGE_REG_P     # 25600 rows per region table (int16-safe)

T = 1024                     # tokens per gather/scatter call (hard max)
A_CHUNKS = 4                 # X gather table chunks (25000 rows)
A_CHUNK_ROWS = N // A_CHUNKS
A_TPC = 10                   # stage-A tiles per (core, chunk) segment
C_SEGS = 2                   # stage-C segments per core (one per region table)
C_TPC = 20                   # stage-C tiles per segment
A_TILES = A_CHUNKS * A_TPC   # 40
C_TILES = C_SEGS * C_TPC     # 40
ND_TILES = NODE_SH_P // 128  # 98
N_ACC = 4                    # parallel scatter chains per accumulator

MM_DT = os.environ.get("BASS_GNN_MM_DT", "f32r")  # f32 | f32r | bf16
STAGES = os.environ.get("BASS_GNN_STAGES", "ZABCD")

_STATE = {}


# ---------------------------------------------------------------- host side
def _wrap16(idx):
    """[n_tiles, T] int -> [n_tiles, 128, T//16] int16 (idx i at partition
    i%16, col i//16; replicated across the 8 groups of 16 partitions)."""
    n_tiles = idx.shape[0]
    t = idx.reshape(n_tiles, T // 16, 16).transpose(0, 2, 1).astype(np.int16)
    return np.ascontiguousarray(np.tile(t, (1, 8, 1)))


def _deal(g, s, n_tiles, junk):
    """Pack one segment's tokens into n_tiles gather/scatter idx tiles.

    g, s: per-token gather idx / scatter row, s sorted ascending.
    Round-robin dealing (token p -> tile p % n_tiles, slot p // n_tiles)
    keeps each tile's scatter rows unique when every row's multiplicity
    <= n_tiles. Pad slots gather row 0 and scatter to junk rows.
    """
    n = len(g)
    if n > n_tiles * T or (n and np.bincount(s).max() > n_tiles):
        return None
    ga = np.zeros((n_tiles, T), np.int64)
    sa = np.empty((n_tiles, T), np.int64)
    sa[:] = junk[None, :]
    p = np.arange(n)
    ga[p % n_tiles, p // n_tiles] = g
    sa[p % n_tiles, p // n_tiles] = s
    return ga, sa


def _route(vertex, edges):
    """Per-core idx tensors for both gather/scatter stages, or None if the
    (astronomically unlikely) static capacities are exceeded."""
    le = edges % EDGE_SH
    xe_reg = le // EDGE_REG                    # region within shard
    xe_shard_row = EDGE_REG_P * xe_reg + (le - EDGE_REG * xe_reg)
    xe_tbl_row = EDGE_REG_P * (edges // EDGE_SH) + (le - EDGE_REG * xe_reg)
    owner_a = edges // EDGE_SH
    chunk_a = vertex // A_CHUNK_ROWS
    owner_c = vertex // NODE_SH

    ar = np.arange(T)
    junk_a = EDGE_REG_P * (ar % 2) + EDGE_REG + (ar // 2) % (EDGE_REG_P - EDGE_REG)
    junk_c = NODE_SH + ar % (NODE_SH_P - NODE_SH)

    ga_all, sa_all, gc_all, sc_all = [], [], [], []
    for m in range(NC):
        ga_m = np.zeros((A_TILES, T), np.int64)
        sa_m = np.empty((A_TILES, T), np.int64)
        sa_m[:] = junk_a[None, :]
        for c in range(A_CHUNKS):
            sel = np.nonzero((owner_a == m) & (chunk_a == c))[0]
            s = xe_shard_row[sel]
            order = np.argsort(s, kind="stable")
            d = _deal(vertex[sel][order] - A_CHUNK_ROWS * c, s[order], A_TPC,
                      junk_a)
            if d is None:
                return None
            ga_m[c * A_TPC:(c + 1) * A_TPC] = d[0]
            sa_m[c * A_TPC:(c + 1) * A_TPC] = d[1]
        gc_m = np.zeros((C_TILES, T), np.int64)
        sc_m = np.empty((C_TILES, T), np.int64)
        sc_m[:] = junk_c[None, :]
        for r in range(C_SEGS):
            sel = np.nonzero((owner_c == m) & (xe_reg == r))[0]
            s = vertex[sel] - NODE_SH * m
            order = np.argsort(s, kind="stable")
            d = _deal(xe_tbl_row[sel][order], s[order], C_TPC, junk_c)
            if d is None:
                return None
            gc_m[r * C_TPC:(r + 1) * C_TPC] = d[0]
            sc_m[r * C_TPC:(r + 1) * C_TPC] = d[1]
        ga_all.append(_wrap16(ga_m))
        sa_all.append(_wrap16(sa_m))
        gc_all.append(_wrap16(gc_m))
        sc_all.append(_wrap16(sc_m))
    return ga_all, sa_all, gc_all, sc_all


def _numpy_fallback(X, vertex, edges, w_b, w_a, w_c, b_b, b_a, b_c):
    Xe = np.zeros((E, F), np.float32)
    np.add.at(Xe, edges, X[vertex])
    Xv2 = np.zeros((N, F), np.float32)
    np.add.at(Xv2, vertex, Xe[edges])
    deg = np.bincount(vertex, minlength=N).astype(np.float32)[:, None]
    Xv = np.concatenate([deg * X, Xv2], axis=1)
    center = Xv @ w_b + b_b
    aXv = np.abs(Xv)
    return (center.astype(np.float32),
            (center - (aXv @ w_a + b_a)).astype(np.float32),
            (center + (aXv @ w_c + b_c)).astype(np.float32))


# ------------------------------------------------------------- bass program
def _build_program():
    from concourse import bacc, tile
    import concourse.mybir as mybir

    f32 = mybir.dt.float32
    mmdt = {"f32": f32, "f32r": mybir.dt.float32r,
            "bf16": mybir.dt.bfloat16}[MM_DT]
    bdt = f32 if MM_DT == "f32r" else mmdt
    i16 = mybir.dt.int16

    nc = bacc.Bacc(None, target_bir_lowering=False, debug=False,
                   num_devices=NC, num_swdge_queues=4)

    xfull = nc.dram_tensor("xfull", [N, F], f32, kind="ExternalInput")
    xshard = nc.dram_tensor("xshard", [NODE_SH, F], f32, kind="ExternalInput")
    ga = nc.dram_tensor("ga", [A_TILES, 128, T // 16], i16, kind="ExternalInput")
    sa = nc.dram_tensor("sa", [A_TILES, 128, T // 16], i16, kind="ExternalInput")
    gc = nc.dram_tensor("gc", [C_TILES, 128, T // 16], i16, kind="ExternalInput")
    sc = nc.dram_tensor("sc", [C_TILES, 128, T // 16], i16, kind="ExternalInput")
    deg = nc.dram_tensor("deg", [ND_TILES, 128, 1], f32, kind="ExternalInput")
    wts_d = {nm: nc.dram_tensor(nm, [F, F], mmdt, kind="ExternalInput")
             for nm in ("wb1", "wb2", "wa1n", "wa2n", "wc1", "wc2")}
    bias_d = {nm: nc.dram_tensor(nm, [1, F], bdt, kind="ExternalInput")
              for nm in ("bias_c", "bias_l", "bias_r")}
    outs = {nm: nc.dram_tensor(nm, [NODE_SH, F], f32, kind="ExternalOutput")
            for nm in ("center", "hl", "hr")}

    xe_acc = [nc.dram_tensor(f"xe_acc{k}", [EDGE_SH_P, F], f32)
              for k in range(N_ACC)]
    xe_sum = nc.dram_tensor("xe_sum", [EDGE_SH_P, F], f32)
    xe_tbl = [nc.dram_tensor(f"xe_tbl{r}", [XE_TBL, F], f32)
              for r in range(2)]
    xv2_acc = [nc.dram_tensor(f"xv2_acc{k}", [NODE_SH_P, F], f32)
               for k in range(N_ACC)]

    eye_d = nc.inline_tensor(np.eye(128, dtype=np.float32), name="eye128")

    def flat(t, lo=None, hi=None):
        ap = t.ap() if lo is None else t[lo:hi, :]
        return ap.rearrange("(p a) f -> p (a f)", p=128)

    with tile.TileContext(nc) as tc:
        with (
            tc.tile_pool(name="cpool", bufs=1) as cpool,
            tc.tile_pool(name="ipool", bufs=8) as ipool,
            tc.tile_pool(name="dpool", bufs=8) as dpool,
            tc.tile_pool(name="spool", bufs=3) as spool,
            tc.tile_pool(name="tpool", bufs=2) as tpool,
            tc.tile_pool(name="opool", bufs=2) as opool,
            tc.tile_pool(name="ps_tr", bufs=1, space="PSUM") as ps_tr,
            tc.tile_pool(name="ps_mm", bufs=2, space="PSUM") as ps_mm,
        ):
            # constants
            zero = cpool.tile([128, 2048], f32)
            nc.vector.memset(zero[:], 0.0)
            ident = cpool.tile([128, 128], f32)
            nc.sync.dma_start(ident[:], eye_d[:])
            ones = cpool.tile([1, F], bdt)
            nc.vector.memset(ones[:], 1.0)
            wts = {}
            for nm, d in wts_d.items():
                wtile = cpool.tile([F, F], mmdt, tag=nm)
                nc.sync.dma_start(wtile[:], d[:])
                wts[nm] = wtile
            biases = {}
            for nm, d in bias_d.items():
                btile = cpool.tile([1, F], bdt, tag=nm)
                nc.sync.dma_start(btile[:], d[:])
                biases[nm] = btile

            # zero the DRAM accumulators
            if "Z" in STAGES:
                for t in xe_acc + xv2_acc:
                    fl = flat(t)
                    for off in range(0, fl.shape[1], 2048):
                        sz = min(2048, fl.shape[1] - off)
                        nc.sync.dma_start(fl[:, off:off + sz], zero[:, :sz])

            # stage A: X[vertex] scatter-added by edge
            for ti in range(A_TILES if "A" in STAGES else 0):
                c = ti // A_TPC
                gt = ipool.tile([128, T // 16], i16, tag="gt")
                st = ipool.tile([128, T // 16], i16, tag="st")
                nc.sync.dma_start(gt[:], ga[ti])
                nc.sync.dma_start(st[:], sa[ti])
                dat = dpool.tile([128, T // 128, F], f32, tag="dat")
                nc.gpsimd.dma_gather(
                    dat[:], xfull[c * A_CHUNK_ROWS:(c + 1) * A_CHUNK_ROWS, :],
                    gt[:], T, T, F, queue_num=ti % 4)
                nc.gpsimd.dma_scatter_add(
                    xe_acc[ti % N_ACC][:], dat[:], st[:], T, T, F,
                    queue_num=(ti + 2) % 4)

            # stage A.5 + B: per edge region, sum the accumulators and
            # all-gather that region's shard (region 0 lands first so
            # stage C region-0 work starts early)
            if "A" in STAGES:
                for r in range(2):
                    lo, hi = r * EDGE_REG_P, (r + 1) * EDGE_REG_P
                    fls = [flat(t, lo, hi) for t in xe_acc]
                    fs = flat(xe_sum, lo, hi)
                    w_tot = fs.shape[1]
                    for off in range(0, w_tot, 1600):
                        sz = min(1600, w_tot - off)
                        ta = spool.tile([128, 1600], f32, tag="sum_a")
                        tb = spool.tile([128, 1600], f32, tag="sum_b")
                        nc.sync.dma_start(ta[:, :sz], fls[0][:, off:off + sz])
                        nc.sync.dma_start(tb[:, :sz], fls[1][:, off:off + sz])
                        nc.vector.tensor_add(ta[:, :sz], ta[:, :sz], tb[:, :sz])
                        tc2 = spool.tile([128, 1600], f32, tag="sum_c")
                        td = spool.tile([128, 1600], f32, tag="sum_d")
                        nc.scalar.dma_start(tc2[:, :sz], fls[2][:, off:off + sz])
                        nc.scalar.dma_start(td[:, :sz], fls[3][:, off:off + sz])
                        nc.vector.tensor_add(tc2[:, :sz], tc2[:, :sz], td[:, :sz])
                        nc.vector.tensor_add(ta[:, :sz], ta[:, :sz], tc2[:, :sz])
                        nc.sync.dma_start(fs[:, off:off + sz], ta[:, :sz])
                    if "B" in STAGES:
                        nc.gpsimd.collective_compute(
                            "AllGather", mybir.AluOpType.bypass,
                            replica_groups=[list(range(NC))],
                            ins=[xe_sum[lo:hi, :].opt()],
                            outs=[xe_tbl[r].ap().opt()],
                        )

            # stage C: Xe[edges] scatter-added by vertex
            for ti in range(C_TILES if "C" in STAGES else 0):
                r = ti // C_TPC
                gt = ipool.tile([128, T // 16], i16, tag="gt")
                st = ipool.tile([128, T // 16], i16, tag="st")
                nc.sync.dma_start(gt[:], gc[ti])
                nc.sync.dma_start(st[:], sc[ti])
                dat = dpool.tile([128, T // 128, F], f32, tag="dat")
                nc.gpsimd.dma_gather(dat[:], xe_tbl[r][:], gt[:], T, T, F,
                                     queue_num=ti % 4)
                nc.gpsimd.dma_scatter_add(
                    xv2_acc[ti % N_ACC][:], dat[:], st[:], T, T, F,
                    queue_num=(ti + 2) % 4)

            # stage D: dense head
            Abs = mybir.ActivationFunctionType.Abs
            Copy = mybir.ActivationFunctionType.Copy
            for nt in range(ND_TILES if "D" in STAGES else 0):
                rows = min(128, NODE_SH - nt * 128)
                r0 = nt * 128
                xt = spool.tile([128, F], f32, tag="xt")
                nc.sync.dma_start(xt[:rows, :], xshard[r0:r0 + rows, :])
                dg = spool.tile([128, 1], f32, tag="dg")
                nc.sync.dma_start(dg[:], deg[nt])
                va = spool.tile([128, F], f32, tag="va")
                vb = spool.tile([128, F], f32, tag="vb")
                vc = spool.tile([128, F], f32, tag="vc")
                vd = spool.tile([128, F], f32, tag="vd")
                nc.scalar.dma_start(va[:], xv2_acc[0][r0:r0 + 128, :])
                nc.scalar.dma_start(vb[:], xv2_acc[1][r0:r0 + 128, :])
                nc.scalar.dma_start(vc[:], xv2_acc[2][r0:r0 + 128, :])
                nc.scalar.dma_start(vd[:], xv2_acc[3][r0:r0 + 128, :])
                nc.vector.tensor_add(va[:], va[:], vb[:])
                nc.vector.tensor_add(vc[:], vc[:], vd[:])
                nc.vector.tensor_add(va[:], va[:], vc[:])

                h1 = spool.tile([128, F], f32, tag="h1")
                nc.scalar.activation(h1[:], xt[:], Copy, scale=dg[:, 0:1])

                h1T_ps = ps_tr.tile([128, F], f32, tag="h1T_ps")
                nc.tensor.transpose(h1T_ps[:], h1[:], ident[:])
                h2T_ps = ps_tr.tile([128, F], f32, tag="h2T_ps")
                nc.tensor.transpose(h2T_ps[:], va[:], ident[:])

                h1T = tpool.tile([128, F], mmdt, tag="h1T")
                nc.vector.tensor_copy(h1T[:], h1T_ps[:])
                h2T = tpool.tile([128, F], mmdt, tag="h2T")
                nc.vector.tensor_copy(h2T[:], h2T_ps[:])
                a1T = tpool.tile([128, F], mmdt, tag="a1T")
                nc.scalar.activation(a1T[:], h1T_ps[:], Abs)
                a2T = tpool.tile([128, F], mmdt, tag="a2T")
                nc.scalar.activation(a2T[:], h2T_ps[:], Abs)

                groups = (
                    ("c_ps", "bias_c", (("h1T", "wb1"), ("h2T", "wb2"))),
                    ("l_ps", "bias_l", (("h1T", "wb1"), ("h2T", "wb2"),
                                        ("a1T", "wa1n"), ("a2T", "wa2n"))),
                    ("r_ps", "bias_r", (("h1T", "wb1"), ("h2T", "wb2"),
                                        ("a1T", "wc1"), ("a2T", "wc2"))),
                )
                lhs = {"h1T": h1T, "h2T": h2T, "a1T": a1T, "a2T": a2T}
                ps_out = {}
                for psname, bias, terms in groups:
                    ps = ps_mm.tile([128, F], f32, tag=psname)
                    nc.tensor.matmul(ps[:], ones[:], biases[bias][:],
                                     start=True, stop=False)
                    for i, (ln, wn) in enumerate(terms):
                        nc.tensor.matmul(ps[:], lhs[ln][:], wts[wn][:],
                                         start=False, stop=(i == len(terms) - 1))
                    ps_out[psname] = ps
                for psname, oname in (("c_ps", "center"), ("l_ps", "hl"),
                                      ("r_ps", "hr")):
                    ot = opool.tile([128, F], f32, tag=f"o_{oname}")
                    nc.vector.tensor_copy(ot[:], ps_out[psname][:])
                    nc.sync.dma_start(outs[oname][r0:r0 + rows, :], ot[:rows, :])

    nc.compile()
    return nc


# ------------------------------------------------------------------- driver
def kernel(X, vertex, edges, X0, n_edges, w_b, w_a, w_c, b_b, b_a, b_c):
    from concourse.bass_utils import run_bass_kernel_spmd
    import ml_dtypes

    X = np.ascontiguousarray(np.asarray(X, dtype=np.float32))
    vertex = np.asarray(vertex).astype(np.int64)
    edges = np.asarray(edges).astype(np.int64)
    w_b = np.asarray(w_b, dtype=np.float32)
    w_a = np.asarray(w_a, dtype=np.float32)
    w_c = np.asarray(w_c, dtype=np.float32)
    b_b = np.asarray(b_b, dtype=np.float32).reshape(1, F)
    b_a = np.asarray(b_a, dtype=np.float32).reshape(1, F)
    b_c = np.asarray(b_c, dtype=np.float32).reshape(1, F)

    routed = _route(vertex, edges)
    if routed is None:
        return _numpy_fallback(X, vertex, edges, w_b, w_a, w_c, b_b, b_a, b_c)
    ga_all, sa_all, gc_all, sc_all = routed

    if "nc" not in _STATE:
        _STATE["nc"] = _build_program()
    nc = _STATE["nc"]

    npmm = ml_dtypes.bfloat16 if MM_DT == "bf16" else np.float32
    deg_full = np.bincount(vertex, minlength=N).astype(np.float32)
    wmats = {
        "wb1": w_b[:F], "wb2": w_b[F:],
        "wa1n": -w_a[:F], "wa2n": -w_a[F:],
        "wc1": w_c[:F], "wc2": w_c[F:],
    }
    bmats = {"bias_c": b_b, "bias_l": b_b - b_a, "bias_r": b_b + b_c}

    in_maps = []
    for m in range(NC):
        dshard = np.zeros(NODE_SH_P, np.float32)
        dshard[:NODE_SH] = deg_full[m * NODE_SH:(m + 1) * NODE_SH]
        im = {
            "xfull": X,
            "xshard": np.ascontiguousarray(X[m * NODE_SH:(m + 1) * NODE_SH]),
            "ga": ga_all[m], "sa": sa_all[m],
            "gc": gc_all[m], "sc": sc_all[m],
            "deg": dshard.reshape(ND_TILES, 128, 1),
        }
        for nm, w in wmats.items():
            im[nm] = np.ascontiguousarray(w.astype(npmm))
        npb = np.float32 if MM_DT == "f32r" else npmm
        for nm, b in bmats.items():
            im[nm] = np.ascontiguousarray(b.astype(npb))
        in_maps.append(im)

    res = run_bass_kernel_spmd(nc, in_maps, list(range(NC)))
    center = np.concatenate([res.results[m]["center"] for m in range(NC)])
    hl = np.concatenate([res.results[m]["hl"] for m in range(NC)])
    hr = np.concatenate([res.results[m]["hr"] for m in range(NC)])
    return center, hl, hr


# revision 8
# speedup vs baseline: 1.0576x; 1.0536x over previous
"""Trainium2 Bass kernel for nn_CrispToFuzzyConv (hypergraph message passing).

Math (see reference):
  Xe   = segment_sum(X[vertex], edges, E)                 # round 1
  Xv   = segment_sum(concat([X[vertex], Xe[edges]]), vertex, N)
       = concat([deg * X, Xv2]),  Xv2 = segment_sum(Xe[edges], vertex, N)
  center = Xv @ w_b + b_b
  HL = center - (|Xv| @ w_a + b_a)
  HR = center + (|Xv| @ w_c + b_c)

Distribution over 8 NeuronCores:
  - round 1 sharded by edge owner: dma_gather rows from replicated X,
    dma_scatter_add into 4 round-robin per-core Xe accumulators (4
    independent WAW chains on 4 SWDGE queues), summed into the padded
    Xe shard
  - 2 AllGather collectives (one per edge region) -> two 25600-row Xe
    tables; region-0 stage-C work starts while region-1 is still landing
  - round 2 sharded by vertex owner: gather from the Xe tables + scatter
    into 4 Xv2 accumulators
  - dense stage per core: deg-scaling (ACT per-partition scale), PE
    transposes, 13 matmuls/tile with bias folded in as K=1 ones x bias

Key hardware constraints baked in (established empirically):
  - dma_gather/dma_scatter_add indices are int16 -> gather tables are
    chunked to <= 32768 rows; <= 1024 indices per call (the SWDGE ring
    rejects 1280+)
  - duplicate scatter rows WITHIN one call race (lost updates) -> tokens
    are dealt round-robin over a segment's tiles so each call's rows are
    unique; pad slots gather row 0 and scatter garbage to junk rows
  - consecutive scatter calls to one tensor serialize (Tile WAW) and
    accumulate exactly -> 4 alternating accumulators give 4 parallel
    chains
  - collective in/out tensors must be Internal, addr_space Local (Shared
    breaks dma_gather reading the output)
  - num_swdge_queues=4 parallelizes Q7 descriptor generation ~3x
"""

import os
import numpy as np

# ---------------------------------------------------------------- constants
N = 100000
E = 50000
NNZ = 300000
F = 128
NC = 8

NODE_SH = N // NC            # 12500
EDGE_SH = E // NC            # 6250
NODE_SH_P = 12544            # 98 * 128 (rows >= 12500 are junk)
EDGE_REG = EDGE_SH // 2      # 3125 real edges per region
EDGE_REG_P = 3200            # 25 * 128 (rows >= 3125 are junk)
EDGE_SH_P = 2 * EDGE_REG_P   # 6400 (regions stacked)
XE_TBL = NC * EDGE_REG_P     # 25600 rows per region table (int16-safe)

T = 1024                     # tokens per gather/scatter call (hard max)
A_CHUNKS = 4                 # X gather table chunks (25000 rows)
A_CHUNK_ROWS = N // A_CHUNKS
A_TPC = 10                   # stage-A tiles per (core, chunk) segment
C_SEGS = 2                   # stage-C segments per core (one per region table)
C_TPC = 20                   # stage-C tiles per segment
A_TILES = A_CHUNKS * A_TPC   # 40
C_TILES = C_SEGS * C_TPC     # 40
ND_TILES = NODE_SH_P // 128  # 98
N_ACC = 2                    # parallel scatter chains per accumulator

MM_DT = os.environ.get("BASS_GNN_MM_DT", "f32r")  # f32 | f32r | bf16
STAGES = os.environ.get("BASS_GNN_STAGES", "ZABCD")

_STATE = {}


# ---------------------------------------------------------------- host side
def _wrap16(idx):
    """[n_tiles, T] int -> [n_tiles, 128, T//16] int16 (idx i at partition
    i%16, col i//16; replicated across the 8 groups of 16 partitions)."""
    n_tiles = idx.shape[0]
    t = idx.reshape(n_tiles, T // 16, 16).transpose(0, 2, 1).astype(np.int16)
    return np.ascontiguousarray(np.tile(t, (1, 8, 1)))


def _deal(g, s, n_tiles, junk):
    """Pack one segment's tokens into n_tiles gather/scatter idx tiles.

    g, s: per-token gather idx / scatter row, s sorted ascending.
    Round-robin dealing (token p -> tile p % n_tiles, slot p // n_tiles)
    keeps each tile's scatter rows unique when every row's multiplicity
    <= n_tiles. Pad slots gather row 0 and scatter to junk rows.
    """
    n = len(g)
    if n > n_tiles * T or (n and np.bincount(s).max() > n_tiles):
        return None
    ga = np.zeros((n_tiles, T), np.int64)
    sa = np.empty((n_tiles, T), np.int64)
    sa[:] = junk[None, :]
    p = np.arange(n)
    ga[p % n_tiles, p // n_tiles] = g
    sa[p % n_tiles, p // n_tiles] = s
    return ga, sa


def _route(vertex, edges):
    """Per-core idx tensors for both gather/scatter stages, or None if the
    (astronomically unlikely) static capacities are exceeded."""
    le = edges % EDGE_SH
    xe_reg = le // EDGE_REG                    # region within shard
    xe_shard_row = EDGE_REG_P * xe_reg + (le - EDGE_REG * xe_reg)
    xe_tbl_row = EDGE_REG_P * (edges // EDGE_SH) + (le - EDGE_REG * xe_reg)
    owner_a = edges // EDGE_SH
    chunk_a = vertex // A_CHUNK_ROWS
    owner_c = vertex // NODE_SH

    ar = np.arange(T)
    junk_a = EDGE_REG_P * (ar % 2) + EDGE_REG + (ar // 2) % (EDGE_REG_P - EDGE_REG)
    junk_c = NODE_SH + ar % (NODE_SH_P - NODE_SH)

    ga_all, sa_all, gc_all, sc_all = [], [], [], []
    for m in range(NC):
        ga_m = np.zeros((A_TILES, T), np.int64)
        sa_m = np.empty((A_TILES, T), np.int64)
        sa_m[:] = junk_a[None, :]
        for c in range(A_CHUNKS):
            sel = np.nonzero((owner_a == m) & (chunk_a == c))[0]
            s = xe_shard_row[sel]
            order = np.argsort(s, kind="stable")
            d = _deal(vertex[sel][order] - A_CHUNK_ROWS * c, s[order], A_TPC,
                      junk_a)
            if d is None:
                return None
            ga_m[c * A_TPC:(c + 1) * A_TPC] = d[0]
            sa_m[c * A_TPC:(c + 1) * A_TPC] = d[1]
        gc_m = np.zeros((C_TILES, T), np.int64)
        sc_m = np.empty((C_TILES, T), np.int64)
        sc_m[:] = junk_c[None, :]
        for r in range(C_SEGS):
            sel = np.nonzero((owner_c == m) & (xe_reg == r))[0]
            s = vertex[sel] - NODE_SH * m
            order = np.argsort(s, kind="stable")
            d = _deal(xe_tbl_row[sel][order], s[order], C_TPC, junk_c)
            if d is None:
                return None
            gc_m[r * C_TPC:(r + 1) * C_TPC] = d[0]
            sc_m[r * C_TPC:(r + 1) * C_TPC] = d[1]
        ga_all.append(_wrap16(ga_m))
        sa_all.append(_wrap16(sa_m))
        gc_all.append(_wrap16(gc_m))
        sc_all.append(_wrap16(sc_m))
    return ga_all, sa_all, gc_all, sc_all


def _numpy_fallback(X, vertex, edges, w_b, w_a, w_c, b_b, b_a, b_c):
    Xe = np.zeros((E, F), np.float32)
    np.add.at(Xe, edges, X[vertex])
    Xv2 = np.zeros((N, F), np.float32)
    np.add.at(Xv2, vertex, Xe[edges])
    deg = np.bincount(vertex, minlength=N).astype(np.float32)[:, None]
    Xv = np.concatenate([deg * X, Xv2], axis=1)
    center = Xv @ w_b + b_b
    aXv = np.abs(Xv)
    return (center.astype(np.float32),
            (center - (aXv @ w_a + b_a)).astype(np.float32),
            (center + (aXv @ w_c + b_c)).astype(np.float32))


# ------------------------------------------------------------- bass program
def _build_program():
    from concourse import bacc, tile
    import concourse.mybir as mybir

    f32 = mybir.dt.float32
    mmdt = {"f32": f32, "f32r": mybir.dt.float32r,
            "bf16": mybir.dt.bfloat16}[MM_DT]
    bdt = f32 if MM_DT == "f32r" else mmdt
    i16 = mybir.dt.int16

    nc = bacc.Bacc(None, target_bir_lowering=False, debug=False,
                   num_devices=NC, num_swdge_queues=4)

    xfull = nc.dram_tensor("xfull", [N, F], f32, kind="ExternalInput")
    xshard = nc.dram_tensor("xshard", [NODE_SH, F], f32, kind="ExternalInput")
    ga = nc.dram_tensor("ga", [A_TILES, 128, T // 16], i16, kind="ExternalInput")
    sa = nc.dram_tensor("sa", [A_TILES, 128, T // 16], i16, kind="ExternalInput")
    gc = nc.dram_tensor("gc", [C_TILES, 128, T // 16], i16, kind="ExternalInput")
    sc = nc.dram_tensor("sc", [C_TILES, 128, T // 16], i16, kind="ExternalInput")
    deg = nc.dram_tensor("deg", [ND_TILES, 128, 1], f32, kind="ExternalInput")
    wts_d = {nm: nc.dram_tensor(nm, [F, F], mmdt, kind="ExternalInput")
             for nm in ("wb1", "wb2", "wa1n", "wa2n", "wc1", "wc2")}
    bias_d = {nm: nc.dram_tensor(nm, [1, F], bdt, kind="ExternalInput")
              for nm in ("bias_c", "bias_l", "bias_r")}
    outs = {nm: nc.dram_tensor(nm, [NODE_SH, F], f32, kind="ExternalOutput")
            for nm in ("center", "hl", "hr")}

    xe_acc = [nc.dram_tensor(f"xe_acc{k}", [EDGE_SH_P, F], f32)
              for k in range(N_ACC)]
    xe_sum = nc.dram_tensor("xe_sum", [EDGE_SH_P, F], f32)
    xe_tbl = [nc.dram_tensor(f"xe_tbl{r}", [XE_TBL, F], f32)
              for r in range(2)]
    xv2_acc = [nc.dram_tensor(f"xv2_acc{k}", [NODE_SH_P, F], f32)
               for k in range(N_ACC)]

    eye_d = nc.inline_tensor(np.eye(128, dtype=np.float32), name="eye128")

    def flat(t, lo=None, hi=None):
        ap = t.ap() if lo is None else t[lo:hi, :]
        return ap.rearrange("(p a) f -> p (a f)", p=128)

    with tile.TileContext(nc) as tc:
        with (
            tc.tile_pool(name="cpool", bufs=1) as cpool,
            tc.tile_pool(name="ipool", bufs=8) as ipool,
            tc.tile_pool(name="dpool", bufs=8) as dpool,
            tc.tile_pool(name="spool", bufs=3) as spool,
            tc.tile_pool(name="tpool", bufs=2) as tpool,
            tc.tile_pool(name="opool", bufs=2) as opool,
            tc.tile_pool(name="ps_tr", bufs=1, space="PSUM") as ps_tr,
            tc.tile_pool(name="ps_mm", bufs=2, space="PSUM") as ps_mm,
        ):
            # constants
            zero = cpool.tile([128, 2048], f32)
            nc.vector.memset(zero[:], 0.0)
            ident = cpool.tile([128, 128], f32)
            nc.sync.dma_start(ident[:], eye_d[:])
            ones = cpool.tile([1, F], bdt)
            nc.vector.memset(ones[:], 1.0)
            wts = {}
            for nm, d in wts_d.items():
                wtile = cpool.tile([F, F], mmdt, tag=nm)
                nc.sync.dma_start(wtile[:], d[:])
                wts[nm] = wtile
            biases = {}
            for nm, d in bias_d.items():
                btile = cpool.tile([1, F], bdt, tag=nm)
                nc.sync.dma_start(btile[:], d[:])
                biases[nm] = btile

            # zero the DRAM accumulators
            if "Z" in STAGES:
                for t in xe_acc + xv2_acc:
                    fl = flat(t)
                    for off in range(0, fl.shape[1], 2048):
                        sz = min(2048, fl.shape[1] - off)
                        nc.sync.dma_start(fl[:, off:off + sz], zero[:, :sz])

            # stage A: X[vertex] scatter-added by edge
            for ti in range(A_TILES if "A" in STAGES else 0):
                c = ti // A_TPC
                gt = ipool.tile([128, T // 16], i16, tag="gt")
                st = ipool.tile([128, T // 16], i16, tag="st")
                nc.sync.dma_start(gt[:], ga[ti])
                nc.sync.dma_start(st[:], sa[ti])
                dat = dpool.tile([128, T // 128, F], f32, tag="dat")
                nc.gpsimd.dma_gather(
                    dat[:], xfull[c * A_CHUNK_ROWS:(c + 1) * A_CHUNK_ROWS, :],
                    gt[:], T, T, F, queue_num=ti % 4)
                nc.gpsimd.dma_scatter_add(
                    xe_acc[ti % N_ACC][:], dat[:], st[:], T, T, F,
                    queue_num=(ti + 2) % 4)

            # stage A.5 + B: per edge region, sum the accumulators and
            # all-gather that region's shard (region 0 lands first so
            # stage C region-0 work starts early)
            if "A" in STAGES:
                for r in range(2):
                    lo, hi = r * EDGE_REG_P, (r + 1) * EDGE_REG_P
                    fls = [flat(t, lo, hi) for t in xe_acc]
                    fs = flat(xe_sum, lo, hi)
                    w_tot = fs.shape[1]
                    for off in range(0, w_tot, 1600):
                        sz = min(1600, w_tot - off)
                        ta = spool.tile([128, 1600], f32, tag="sum_a")
                        tb = spool.tile([128, 1600], f32, tag="sum_b")
                        nc.sync.dma_start(ta[:, :sz], fls[0][:, off:off + sz])
                        nc.scalar.dma_start(tb[:, :sz], fls[1][:, off:off + sz])
                        nc.vector.tensor_add(ta[:, :sz], ta[:, :sz], tb[:, :sz])
                        nc.sync.dma_start(fs[:, off:off + sz], ta[:, :sz])
                    if "B" in STAGES:
                        nc.gpsimd.collective_compute(
                            "AllGather", mybir.AluOpType.bypass,
                            replica_groups=[list(range(NC))],
                            ins=[xe_sum[lo:hi, :].opt()],
                            outs=[xe_tbl[r].ap().opt()],
                        )

            # stage C: Xe[edges] scatter-added by vertex
            for ti in range(C_TILES if "C" in STAGES else 0):
                r = ti // C_TPC
                gt = ipool.tile([128, T // 16], i16, tag="gt")
                st = ipool.tile([128, T // 16], i16, tag="st")
                nc.sync.dma_start(gt[:], gc[ti])
                nc.sync.dma_start(st[:], sc[ti])
                dat = dpool.tile([128, T // 128, F], f32, tag="dat")
                nc.gpsimd.dma_gather(dat[:], xe_tbl[r][:], gt[:], T, T, F,
                                     queue_num=ti % 4)
                nc.gpsimd.dma_scatter_add(
                    xv2_acc[ti % N_ACC][:], dat[:], st[:], T, T, F,
                    queue_num=(ti + 2) % 4)

            # stage D: dense head
            Abs = mybir.ActivationFunctionType.Abs
            Copy = mybir.ActivationFunctionType.Copy
            for nt in range(ND_TILES if "D" in STAGES else 0):
                rows = min(128, NODE_SH - nt * 128)
                r0 = nt * 128
                xt = spool.tile([128, F], f32, tag="xt")
                nc.sync.dma_start(xt[:rows, :], xshard[r0:r0 + rows, :])
                dg = spool.tile([128, 1], f32, tag="dg")
                nc.sync.dma_start(dg[:], deg[nt])
                va = spool.tile([128, F], f32, tag="va")
                vb = spool.tile([128, F], f32, tag="vb")
                nc.scalar.dma_start(va[:], xv2_acc[0][r0:r0 + 128, :])
                nc.sync.dma_start(vb[:], xv2_acc[1][r0:r0 + 128, :])
                nc.vector.tensor_add(va[:], va[:], vb[:])

                h1 = spool.tile([128, F], f32, tag="h1")
                nc.vector.tensor_scalar_mul(h1[:], xt[:], dg[:, 0:1])

                h1T_ps = ps_tr.tile([128, F], f32, tag="h1T_ps")
                nc.tensor.transpose(h1T_ps[:], h1[:], ident[:])
                h2T_ps = ps_tr.tile([128, F], f32, tag="h2T_ps")
                nc.tensor.transpose(h2T_ps[:], va[:], ident[:])

                h1T = tpool.tile([128, F], mmdt, tag="h1T")
                nc.vector.tensor_copy(h1T[:], h1T_ps[:])
                h2T = tpool.tile([128, F], mmdt, tag="h2T")
                nc.vector.tensor_copy(h2T[:], h2T_ps[:])
                a1T = tpool.tile([128, F], mmdt, tag="a1T")
                nc.scalar.activation(a1T[:], h1T_ps[:], Abs)
                a2T = tpool.tile([128, F], mmdt, tag="a2T")
                nc.scalar.activation(a2T[:], h2T_ps[:], Abs)

                groups = (
                    ("c_ps", "bias_c", (("h1T", "wb1"), ("h2T", "wb2"))),
                    ("l_ps", "bias_l", (("h1T", "wb1"), ("h2T", "wb2"),
                                        ("a1T", "wa1n"), ("a2T", "wa2n"))),
                    ("r_ps", "bias_r", (("h1T", "wb1"), ("h2T", "wb2"),
                                        ("a1T", "wc1"), ("a2T", "wc2"))),
                )
                lhs = {"h1T": h1T, "h2T": h2T, "a1T": a1T, "a2T": a2T}
                ps_out = {}
                for psname, bias, terms in groups:
                    ps = ps_mm.tile([128, F], f32, tag=psname)
                    nc.tensor.matmul(ps[:], ones[:], biases[bias][:],
                                     start=True, stop=False)
                    for i, (ln, wn) in enumerate(terms):
                        nc.tensor.matmul(ps[:], lhs[ln][:], wts[wn][:],
                                         start=False, stop=(i == len(terms) - 1))
                    ps_out[psname] = ps
                for psname, oname in (("c_ps", "center"), ("l_ps", "hl"),
                                      ("r_ps", "hr")):
                    ot = opool.tile([128, F], f32, tag=f"o_{oname}")
                    nc.vector.tensor_copy(ot[:], ps_out[psname][:])
                    nc.sync.dma_start(outs[oname][r0:r0 + rows, :], ot[:rows, :])

    nc.compile()
    return nc


# ------------------------------------------------------------------- driver
def kernel(X, vertex, edges, X0, n_edges, w_b, w_a, w_c, b_b, b_a, b_c):
    from concourse.bass_utils import run_bass_kernel_spmd
    import ml_dtypes

    X = np.ascontiguousarray(np.asarray(X, dtype=np.float32))
    vertex = np.asarray(vertex).astype(np.int64)
    edges = np.asarray(edges).astype(np.int64)
    w_b = np.asarray(w_b, dtype=np.float32)
    w_a = np.asarray(w_a, dtype=np.float32)
    w_c = np.asarray(w_c, dtype=np.float32)
    b_b = np.asarray(b_b, dtype=np.float32).reshape(1, F)
    b_a = np.asarray(b_a, dtype=np.float32).reshape(1, F)
    b_c = np.asarray(b_c, dtype=np.float32).reshape(1, F)

    routed = _route(vertex, edges)
    if routed is None:
        return _numpy_fallback(X, vertex, edges, w_b, w_a, w_c, b_b, b_a, b_c)
    ga_all, sa_all, gc_all, sc_all = routed

    if "nc" not in _STATE:
        _STATE["nc"] = _build_program()
    nc = _STATE["nc"]

    npmm = ml_dtypes.bfloat16 if MM_DT == "bf16" else np.float32
    deg_full = np.bincount(vertex, minlength=N).astype(np.float32)
    wmats = {
        "wb1": w_b[:F], "wb2": w_b[F:],
        "wa1n": -w_a[:F], "wa2n": -w_a[F:],
        "wc1": w_c[:F], "wc2": w_c[F:],
    }
    bmats = {"bias_c": b_b, "bias_l": b_b - b_a, "bias_r": b_b + b_c}

    in_maps = []
    for m in range(NC):
        dshard = np.zeros(NODE_SH_P, np.float32)
        dshard[:NODE_SH] = deg_full[m * NODE_SH:(m + 1) * NODE_SH]
        im = {
            "xfull": X,
            "xshard": np.ascontiguousarray(X[m * NODE_SH:(m + 1) * NODE_SH]),
            "ga": ga_all[m], "sa": sa_all[m],
            "gc": gc_all[m], "sc": sc_all[m],
            "deg": dshard.reshape(ND_TILES, 128, 1),
        }
        for nm, w in wmats.items():
            im[nm] = np.ascontiguousarray(w.astype(npmm))
        npb = np.float32 if MM_DT == "f32r" else npmm
        for nm, b in bmats.items():
            im[nm] = np.ascontiguousarray(b.astype(npb))
        in_maps.append(im)

    res = run_bass_kernel_spmd(nc, in_maps, list(range(NC)))
    center = np.concatenate([res.results[m]["center"] for m in range(NC)])
    hl = np.concatenate([res.results[m]["hl"] for m in range(NC)])
    hr = np.concatenate([res.results[m]["hr"] for m in range(NC)])
    return center, hl, hr


# revision 9
# speedup vs baseline: 1.0582x; 1.0005x over previous
"""Trainium2 Bass kernel for nn_CrispToFuzzyConv (hypergraph message passing).

Math (see reference):
  Xe   = segment_sum(X[vertex], edges, E)                 # round 1
  Xv   = segment_sum(concat([X[vertex], Xe[edges]]), vertex, N)
       = concat([deg * X, Xv2]),  Xv2 = segment_sum(Xe[edges], vertex, N)
  center = Xv @ w_b + b_b
  HL = center - (|Xv| @ w_a + b_a)
  HR = center + (|Xv| @ w_c + b_c)

Distribution over 8 NeuronCores:
  - round 1 sharded by edge owner: dma_gather rows from replicated X,
    dma_scatter_add into 4 round-robin per-core Xe accumulators (4
    independent WAW chains on 4 SWDGE queues), summed into the padded
    Xe shard
  - 2 AllGather collectives (one per edge region) -> two 25600-row Xe
    tables; region-0 stage-C work starts while region-1 is still landing
  - round 2 sharded by vertex owner: gather from the Xe tables + scatter
    into 4 Xv2 accumulators
  - dense stage per core: deg-scaling (ACT per-partition scale), PE
    transposes, 13 matmuls/tile with bias folded in as K=1 ones x bias

Key hardware constraints baked in (established empirically):
  - dma_gather/dma_scatter_add indices are int16 -> gather tables are
    chunked to <= 32768 rows; <= 1024 indices per call (the SWDGE ring
    rejects 1280+)
  - duplicate scatter rows WITHIN one call race (lost updates) -> tokens
    are dealt round-robin over a segment's tiles so each call's rows are
    unique; pad slots gather row 0 and scatter garbage to junk rows
  - consecutive scatter calls to one tensor serialize (Tile WAW) and
    accumulate exactly -> 4 alternating accumulators give 4 parallel
    chains
  - collective in/out tensors must be Internal, addr_space Local (Shared
    breaks dma_gather reading the output)
  - num_swdge_queues=4 parallelizes Q7 descriptor generation ~3x
"""

import os
import numpy as np

# ---------------------------------------------------------------- constants
N = 100000
E = 50000
NNZ = 300000
F = 128
NC = 8

NODE_SH = N // NC            # 12500
EDGE_SH = E // NC            # 6250
NODE_SH_P = 12544            # 98 * 128 (rows >= 12500 are junk)
EDGE_REG = EDGE_SH // 2      # 3125 real edges per region
EDGE_REG_P = 3200            # 25 * 128 (rows >= 3125 are junk)
EDGE_SH_P = 2 * EDGE_REG_P   # 6400 (regions stacked)
XE_TBL = NC * EDGE_REG_P     # 25600 rows per region table (int16-safe)

T = 1024                     # tokens per gather/scatter call (hard max)
A_CHUNKS = 4                 # X gather table chunks (25000 rows)
A_CHUNK_ROWS = N // A_CHUNKS
A_TPC = 10                   # stage-A tiles per (core, chunk) segment
C_SEGS = 2                   # stage-C segments per core (one per region table)
C_TPC = 20                   # stage-C tiles per segment
A_TILES = A_CHUNKS * A_TPC   # 40
C_TILES = C_SEGS * C_TPC     # 40
ND_TILES = NODE_SH_P // 128  # 98
N_ACC = 2                    # parallel scatter chains per accumulator

MM_DT = os.environ.get("BASS_GNN_MM_DT", "f32r")  # f32 | f32r | bf16
STAGES = os.environ.get("BASS_GNN_STAGES", "ZABCD")

_STATE = {}


# ---------------------------------------------------------------- host side
def _wrap16(idx):
    """[n_tiles, T] int -> [n_tiles, 128, T//16] int16 (idx i at partition
    i%16, col i//16; replicated across the 8 groups of 16 partitions)."""
    n_tiles = idx.shape[0]
    t = idx.reshape(n_tiles, T // 16, 16).transpose(0, 2, 1).astype(np.int16)
    return np.ascontiguousarray(np.tile(t, (1, 8, 1)))


def _deal(g, s, n_tiles, junk):
    """Pack one segment's tokens into n_tiles gather/scatter idx tiles.

    g, s: per-token gather idx / scatter row, s sorted ascending.
    Round-robin dealing (token p -> tile p % n_tiles, slot p // n_tiles)
    keeps each tile's scatter rows unique when every row's multiplicity
    <= n_tiles. Pad slots gather row 0 and scatter to junk rows.
    """
    n = len(g)
    if n > n_tiles * T or (n and np.bincount(s).max() > n_tiles):
        return None
    ga = np.zeros((n_tiles, T), np.int64)
    sa = np.empty((n_tiles, T), np.int64)
    sa[:] = junk[None, :]
    p = np.arange(n)
    ga[p % n_tiles, p // n_tiles] = g
    sa[p % n_tiles, p // n_tiles] = s
    return ga, sa


def _route(vertex, edges):
    """Per-core idx tensors for both gather/scatter stages, or None if the
    (astronomically unlikely) static capacities are exceeded."""
    le = edges % EDGE_SH
    xe_reg = le // EDGE_REG                    # region within shard
    xe_shard_row = EDGE_REG_P * xe_reg + (le - EDGE_REG * xe_reg)
    xe_tbl_row = EDGE_REG_P * (edges // EDGE_SH) + (le - EDGE_REG * xe_reg)
    owner_a = edges // EDGE_SH
    chunk_a = vertex // A_CHUNK_ROWS
    owner_c = vertex // NODE_SH

    ar = np.arange(T)
    junk_a = EDGE_REG_P * (ar % 2) + EDGE_REG + (ar // 2) % (EDGE_REG_P - EDGE_REG)
    junk_c = NODE_SH + ar % (NODE_SH_P - NODE_SH)

    ga_all, sa_all, gc_all, sc_all = [], [], [], []
    for m in range(NC):
        ga_m = np.zeros((A_TILES, T), np.int64)
        sa_m = np.empty((A_TILES, T), np.int64)
        sa_m[:] = junk_a[None, :]
        for c in range(A_CHUNKS):
            sel = np.nonzero((owner_a == m) & (chunk_a == c))[0]
            s = xe_shard_row[sel]
            order = np.argsort(s, kind="stable")
            d = _deal(vertex[sel][order] - A_CHUNK_ROWS * c, s[order], A_TPC,
                      junk_a)
            if d is None:
                return None
            ga_m[c * A_TPC:(c + 1) * A_TPC] = d[0]
            sa_m[c * A_TPC:(c + 1) * A_TPC] = d[1]
        gc_m = np.zeros((C_TILES, T), np.int64)
        sc_m = np.empty((C_TILES, T), np.int64)
        sc_m[:] = junk_c[None, :]
        for r in range(C_SEGS):
            sel = np.nonzero((owner_c == m) & (xe_reg == r))[0]
            s = vertex[sel] - NODE_SH * m
            order = np.argsort(s, kind="stable")
            d = _deal(xe_tbl_row[sel][order], s[order], C_TPC, junk_c)
            if d is None:
                return None
            gc_m[r * C_TPC:(r + 1) * C_TPC] = d[0]
            sc_m[r * C_TPC:(r + 1) * C_TPC] = d[1]
        ga_all.append(_wrap16(ga_m))
        sa_all.append(_wrap16(sa_m))
        gc_all.append(_wrap16(gc_m))
        sc_all.append(_wrap16(sc_m))
    return ga_all, sa_all, gc_all, sc_all


def _numpy_fallback(X, vertex, edges, w_b, w_a, w_c, b_b, b_a, b_c):
    Xe = np.zeros((E, F), np.float32)
    np.add.at(Xe, edges, X[vertex])
    Xv2 = np.zeros((N, F), np.float32)
    np.add.at(Xv2, vertex, Xe[edges])
    deg = np.bincount(vertex, minlength=N).astype(np.float32)[:, None]
    Xv = np.concatenate([deg * X, Xv2], axis=1)
    center = Xv @ w_b + b_b
    aXv = np.abs(Xv)
    return (center.astype(np.float32),
            (center - (aXv @ w_a + b_a)).astype(np.float32),
            (center + (aXv @ w_c + b_c)).astype(np.float32))


# ------------------------------------------------------------- bass program
def _build_program():
    from concourse import bacc, tile
    import concourse.mybir as mybir

    f32 = mybir.dt.float32
    mmdt = {"f32": f32, "f32r": mybir.dt.float32r,
            "bf16": mybir.dt.bfloat16}[MM_DT]
    bdt = f32 if MM_DT == "f32r" else mmdt
    i16 = mybir.dt.int16

    nc = bacc.Bacc(None, target_bir_lowering=False, debug=False,
                   num_devices=NC, num_swdge_queues=4)

    xfull = nc.dram_tensor("xfull", [N, F], f32, kind="ExternalInput")
    xshard = nc.dram_tensor("xshard", [NODE_SH, F], f32, kind="ExternalInput")
    ga = nc.dram_tensor("ga", [A_TILES, 128, T // 16], i16, kind="ExternalInput")
    sa = nc.dram_tensor("sa", [A_TILES, 128, T // 16], i16, kind="ExternalInput")
    gc = nc.dram_tensor("gc", [C_TILES, 128, T // 16], i16, kind="ExternalInput")
    sc = nc.dram_tensor("sc", [C_TILES, 128, T // 16], i16, kind="ExternalInput")
    deg = nc.dram_tensor("deg", [ND_TILES, 128, 1], f32, kind="ExternalInput")
    wts_d = {nm: nc.dram_tensor(nm, [F, F], mmdt, kind="ExternalInput")
             for nm in ("wb1", "wb2", "wa1n", "wa2n", "wc1", "wc2")}
    bias_d = {nm: nc.dram_tensor(nm, [1, F], bdt, kind="ExternalInput")
              for nm in ("bias_c", "bias_l", "bias_r")}
    outs = {nm: nc.dram_tensor(nm, [NODE_SH, F], f32, kind="ExternalOutput")
            for nm in ("center", "hl", "hr")}

    xe_acc = [nc.dram_tensor(f"xe_acc{k}", [EDGE_SH_P, F], f32)
              for k in range(N_ACC)]
    xe_sum = nc.dram_tensor("xe_sum", [EDGE_SH_P, F], f32)
    xe_tbl = [nc.dram_tensor(f"xe_tbl{r}", [XE_TBL, F], f32)
              for r in range(2)]
    xv2_acc = [nc.dram_tensor(f"xv2_acc{k}", [NODE_SH_P, F], f32)
               for k in range(N_ACC)]

    eye_d = nc.inline_tensor(np.eye(128, dtype=np.float32), name="eye128")

    def flat(t, lo=None, hi=None):
        ap = t.ap() if lo is None else t[lo:hi, :]
        return ap.rearrange("(p a) f -> p (a f)", p=128)

    with tile.TileContext(nc) as tc:
        with (
            tc.tile_pool(name="cpool", bufs=1) as cpool,
            tc.tile_pool(name="ipool", bufs=8) as ipool,
            tc.tile_pool(name="dpool", bufs=8) as dpool,
            tc.tile_pool(name="spool", bufs=3) as spool,
            tc.tile_pool(name="tpool", bufs=2) as tpool,
            tc.tile_pool(name="opool", bufs=2) as opool,
            tc.tile_pool(name="ps_tr", bufs=1, space="PSUM") as ps_tr,
            tc.tile_pool(name="ps_mm", bufs=2, space="PSUM") as ps_mm,
        ):
            # constants
            zero = cpool.tile([128, 2048], f32)
            nc.vector.memset(zero[:], 0.0)
            ident = cpool.tile([128, 128], f32)
            nc.sync.dma_start(ident[:], eye_d[:])
            ones = cpool.tile([1, F], bdt)
            nc.vector.memset(ones[:], 1.0)
            wts = {}
            for nm, d in wts_d.items():
                wtile = cpool.tile([F, F], mmdt, tag=nm)
                nc.sync.dma_start(wtile[:], d[:])
                wts[nm] = wtile
            biases = {}
            bias_bc = {}
            for nm, d in bias_d.items():
                btile = cpool.tile([1, F], bdt, tag=nm)
                nc.sync.dma_start(btile[:], d[:])
                biases[nm] = btile
            for nm in bias_d:
                bps = ps_mm.tile([128, F], f32, tag="c_ps")
                nc.tensor.matmul(bps[:], ones[:], biases[nm][:],
                                 start=True, stop=True)
                bct = cpool.tile([128, F], f32, tag=f"bc_{nm}")
                nc.vector.tensor_copy(bct[:], bps[:])
                bias_bc[nm] = bct

            # zero the DRAM accumulators
            if "Z" in STAGES:
                for t in xe_acc + xv2_acc:
                    fl = flat(t)
                    for off in range(0, fl.shape[1], 2048):
                        sz = min(2048, fl.shape[1] - off)
                        nc.sync.dma_start(fl[:, off:off + sz], zero[:, :sz])

            # stage A: X[vertex] scatter-added by edge
            for ti in range(A_TILES if "A" in STAGES else 0):
                c = ti // A_TPC
                gt = ipool.tile([128, T // 16], i16, tag="gt")
                st = ipool.tile([128, T // 16], i16, tag="st")
                nc.sync.dma_start(gt[:], ga[ti])
                nc.sync.dma_start(st[:], sa[ti])
                dat = dpool.tile([128, T // 128, F], f32, tag="dat")
                nc.gpsimd.dma_gather(
                    dat[:], xfull[c * A_CHUNK_ROWS:(c + 1) * A_CHUNK_ROWS, :],
                    gt[:], T, T, F, queue_num=ti % 4)
                nc.gpsimd.dma_scatter_add(
                    xe_acc[ti % N_ACC][:], dat[:], st[:], T, T, F,
                    queue_num=(ti + 2) % 4)

            # stage A.5 + B: per edge region, sum the accumulators and
            # all-gather that region's shard (region 0 lands first so
            # stage C region-0 work starts early)
            if "A" in STAGES:
                for r in range(2):
                    lo, hi = r * EDGE_REG_P, (r + 1) * EDGE_REG_P
                    fls = [flat(t, lo, hi) for t in xe_acc]
                    fs = flat(xe_sum, lo, hi)
                    w_tot = fs.shape[1]
                    for off in range(0, w_tot, 1600):
                        sz = min(1600, w_tot - off)
                        ta = spool.tile([128, 1600], f32, tag="sum_a")
                        tb = spool.tile([128, 1600], f32, tag="sum_b")
                        nc.sync.dma_start(ta[:, :sz], fls[0][:, off:off + sz])
                        nc.scalar.dma_start(tb[:, :sz], fls[1][:, off:off + sz])
                        nc.vector.tensor_add(ta[:, :sz], ta[:, :sz], tb[:, :sz])
                        nc.sync.dma_start(fs[:, off:off + sz], ta[:, :sz])
                    if "B" in STAGES:
                        nc.gpsimd.collective_compute(
                            "AllGather", mybir.AluOpType.bypass,
                            replica_groups=[list(range(NC))],
                            ins=[xe_sum[lo:hi, :].opt()],
                            outs=[xe_tbl[r].ap().opt()],
                        )

            # stage C: Xe[edges] scatter-added by vertex
            for ti in range(C_TILES if "C" in STAGES else 0):
                r = ti // C_TPC
                gt = ipool.tile([128, T // 16], i16, tag="gt")
                st = ipool.tile([128, T // 16], i16, tag="st")
                nc.sync.dma_start(gt[:], gc[ti])
                nc.sync.dma_start(st[:], sc[ti])
                dat = dpool.tile([128, T // 128, F], f32, tag="dat")
                nc.gpsimd.dma_gather(dat[:], xe_tbl[r][:], gt[:], T, T, F,
                                     queue_num=ti % 4)
                nc.gpsimd.dma_scatter_add(
                    xv2_acc[ti % N_ACC][:], dat[:], st[:], T, T, F,
                    queue_num=(ti + 2) % 4)

            # stage D: dense head
            Abs = mybir.ActivationFunctionType.Abs
            Copy = mybir.ActivationFunctionType.Copy
            for nt in range(ND_TILES if "D" in STAGES else 0):
                rows = min(128, NODE_SH - nt * 128)
                r0 = nt * 128
                xt = spool.tile([128, F], f32, tag="xt")
                nc.sync.dma_start(xt[:rows, :], xshard[r0:r0 + rows, :])
                dg = spool.tile([128, 1], f32, tag="dg")
                nc.sync.dma_start(dg[:], deg[nt])
                va = spool.tile([128, F], f32, tag="va")
                vb = spool.tile([128, F], f32, tag="vb")
                nc.scalar.dma_start(va[:], xv2_acc[0][r0:r0 + 128, :])
                nc.sync.dma_start(vb[:], xv2_acc[1][r0:r0 + 128, :])
                nc.vector.tensor_add(va[:], va[:], vb[:])

                h1 = spool.tile([128, F], f32, tag="h1")
                nc.vector.tensor_scalar_mul(h1[:], xt[:], dg[:, 0:1])

                h1T_ps = ps_tr.tile([128, F], f32, tag="h1T_ps")
                nc.tensor.transpose(h1T_ps[:], h1[:], ident[:])
                h2T_ps = ps_tr.tile([128, F], f32, tag="h2T_ps")
                nc.tensor.transpose(h2T_ps[:], va[:], ident[:])

                h1T = tpool.tile([128, F], mmdt, tag="h1T")
                nc.vector.tensor_copy(h1T[:], h1T_ps[:])
                h2T = tpool.tile([128, F], mmdt, tag="h2T")
                nc.vector.tensor_copy(h2T[:], h2T_ps[:])
                a1T = tpool.tile([128, F], mmdt, tag="a1T")
                nc.scalar.activation(a1T[:], h1T_ps[:], Abs)
                a2T = tpool.tile([128, F], mmdt, tag="a2T")
                nc.scalar.activation(a2T[:], h2T_ps[:], Abs)

                groups = (
                    ("c_ps", "bias_c", (("h1T", "wb1"), ("h2T", "wb2"))),
                    ("l_ps", "bias_l", (("h1T", "wb1"), ("h2T", "wb2"),
                                        ("a1T", "wa1n"), ("a2T", "wa2n"))),
                    ("r_ps", "bias_r", (("h1T", "wb1"), ("h2T", "wb2"),
                                        ("a1T", "wc1"), ("a2T", "wc2"))),
                )
                lhs = {"h1T": h1T, "h2T": h2T, "a1T": a1T, "a2T": a2T}
                ps_out = {}
                grp_bias = {}
                for psname, bias, terms in groups:
                    ps = ps_mm.tile([128, F], f32, tag=psname)
                    for i, (ln, wn) in enumerate(terms):
                        nc.tensor.matmul(ps[:], lhs[ln][:], wts[wn][:],
                                         start=(i == 0), stop=(i == len(terms) - 1))
                    ps_out[psname] = ps
                    grp_bias[psname] = bias_bc[bias]
                for psname, oname in (("c_ps", "center"), ("l_ps", "hl"),
                                      ("r_ps", "hr")):
                    ot = opool.tile([128, F], f32, tag=f"o_{oname}")
                    nc.vector.tensor_add(ot[:], grp_bias[psname][:],
                                         ps_out[psname][:])
                    nc.sync.dma_start(outs[oname][r0:r0 + rows, :], ot[:rows, :])

    nc.compile()
    return nc


# ------------------------------------------------------------------- driver
def kernel(X, vertex, edges, X0, n_edges, w_b, w_a, w_c, b_b, b_a, b_c):
    from concourse.bass_utils import run_bass_kernel_spmd
    import ml_dtypes

    X = np.ascontiguousarray(np.asarray(X, dtype=np.float32))
    vertex = np.asarray(vertex).astype(np.int64)
    edges = np.asarray(edges).astype(np.int64)
    w_b = np.asarray(w_b, dtype=np.float32)
    w_a = np.asarray(w_a, dtype=np.float32)
    w_c = np.asarray(w_c, dtype=np.float32)
    b_b = np.asarray(b_b, dtype=np.float32).reshape(1, F)
    b_a = np.asarray(b_a, dtype=np.float32).reshape(1, F)
    b_c = np.asarray(b_c, dtype=np.float32).reshape(1, F)

    routed = _route(vertex, edges)
    if routed is None:
        return _numpy_fallback(X, vertex, edges, w_b, w_a, w_c, b_b, b_a, b_c)
    ga_all, sa_all, gc_all, sc_all = routed

    if "nc" not in _STATE:
        _STATE["nc"] = _build_program()
    nc = _STATE["nc"]

    npmm = ml_dtypes.bfloat16 if MM_DT == "bf16" else np.float32
    deg_full = np.bincount(vertex, minlength=N).astype(np.float32)
    wmats = {
        "wb1": w_b[:F], "wb2": w_b[F:],
        "wa1n": -w_a[:F], "wa2n": -w_a[F:],
        "wc1": w_c[:F], "wc2": w_c[F:],
    }
    bmats = {"bias_c": b_b, "bias_l": b_b - b_a, "bias_r": b_b + b_c}

    in_maps = []
    for m in range(NC):
        dshard = np.zeros(NODE_SH_P, np.float32)
        dshard[:NODE_SH] = deg_full[m * NODE_SH:(m + 1) * NODE_SH]
        im = {
            "xfull": X,
            "xshard": np.ascontiguousarray(X[m * NODE_SH:(m + 1) * NODE_SH]),
            "ga": ga_all[m], "sa": sa_all[m],
            "gc": gc_all[m], "sc": sc_all[m],
            "deg": dshard.reshape(ND_TILES, 128, 1),
        }
        for nm, w in wmats.items():
            im[nm] = np.ascontiguousarray(w.astype(npmm))
        npb = np.float32 if MM_DT == "f32r" else npmm
        for nm, b in bmats.items():
            im[nm] = np.ascontiguousarray(b.astype(npb))
        in_maps.append(im)

    res = run_bass_kernel_spmd(nc, in_maps, list(range(NC)))
    center = np.concatenate([res.results[m]["center"] for m in range(NC)])
    hl = np.concatenate([res.results[m]["hl"] for m in range(NC)])
    hr = np.concatenate([res.results[m]["hr"] for m in range(NC)])
    return center, hl, hr


# revision 10
# speedup vs baseline: 1.0710x; 1.0120x over previous
"""Trainium2 Bass kernel for nn_CrispToFuzzyConv (hypergraph message passing).

Math (see reference):
  Xe   = segment_sum(X[vertex], edges, E)                 # round 1
  Xv   = segment_sum(concat([X[vertex], Xe[edges]]), vertex, N)
       = concat([deg * X, Xv2]),  Xv2 = segment_sum(Xe[edges], vertex, N)
  center = Xv @ w_b + b_b
  HL = center - (|Xv| @ w_a + b_a)
  HR = center + (|Xv| @ w_c + b_c)

Distribution over 8 NeuronCores:
  - round 1 sharded by edge owner: dma_gather rows from replicated X,
    dma_scatter_add into 4 round-robin per-core Xe accumulators (4
    independent WAW chains on 4 SWDGE queues), summed into the padded
    Xe shard
  - 2 AllGather collectives (one per edge region) -> two 25600-row Xe
    tables; region-0 stage-C work starts while region-1 is still landing
  - round 2 sharded by vertex owner: gather from the Xe tables + scatter
    into 4 Xv2 accumulators
  - dense stage per core: deg-scaling (ACT per-partition scale), PE
    transposes, 13 matmuls/tile with bias folded in as K=1 ones x bias

Key hardware constraints baked in (established empirically):
  - dma_gather/dma_scatter_add indices are int16 -> gather tables are
    chunked to <= 32768 rows; <= 1024 indices per call (the SWDGE ring
    rejects 1280+)
  - duplicate scatter rows WITHIN one call race (lost updates) -> tokens
    are dealt round-robin over a segment's tiles so each call's rows are
    unique; pad slots gather row 0 and scatter garbage to junk rows
  - consecutive scatter calls to one tensor serialize (Tile WAW) and
    accumulate exactly -> 4 alternating accumulators give 4 parallel
    chains
  - collective in/out tensors must be Internal, addr_space Local (Shared
    breaks dma_gather reading the output)
  - num_swdge_queues=4 parallelizes Q7 descriptor generation ~3x
"""

import os
import numpy as np

# ---------------------------------------------------------------- constants
N = 100000
E = 50000
NNZ = 300000
F = 128
NC = 8

NODE_SH = N // NC            # 12500
EDGE_SH = E // NC            # 6250
NODE_SH_P = 12544            # 98 * 128 (rows >= 12500 are junk)
EDGE_REG = EDGE_SH // 2      # 3125 real edges per region
EDGE_REG_P = 3200            # 25 * 128 (rows >= 3125 are junk)
EDGE_SH_P = 2 * EDGE_REG_P   # 6400 (regions stacked)
XE_TBL = NC * EDGE_REG_P     # 25600 rows per region table (int16-safe)

T = 1024                     # tokens per gather/scatter call (hard max)
A_CHUNKS = 4                 # X gather table chunks (25000 rows)
A_CHUNK_ROWS = N // A_CHUNKS
A_TPC = 10                   # stage-A tiles per (core, chunk) segment
C_SEGS = 2                   # stage-C segments per core (one per region table)
C_TPC = 20                   # stage-C tiles per segment
A_TILES = A_CHUNKS * A_TPC   # 40
C_TILES = C_SEGS * C_TPC     # 40
ND_TILES = NODE_SH_P // 128  # 98
N_ACC = 2                    # parallel scatter chains per accumulator

MM_DT = os.environ.get("BASS_GNN_MM_DT", "f32r")  # f32 | f32r | bf16
STAGES = os.environ.get("BASS_GNN_STAGES", "ZABCD")

_STATE = {}


# ---------------------------------------------------------------- host side
def _wrap16(idx):
    """[n_tiles, T] int -> [n_tiles, 128, T//16] int16 (idx i at partition
    i%16, col i//16; replicated across the 8 groups of 16 partitions)."""
    n_tiles = idx.shape[0]
    t = idx.reshape(n_tiles, T // 16, 16).transpose(0, 2, 1).astype(np.int16)
    return np.ascontiguousarray(np.tile(t, (1, 8, 1)))


def _deal(g, s, n_tiles, junk):
    """Pack one segment's tokens into n_tiles gather/scatter idx tiles.

    g, s: per-token gather idx / scatter row, s sorted ascending.
    Round-robin dealing (token p -> tile p % n_tiles, slot p // n_tiles)
    keeps each tile's scatter rows unique when every row's multiplicity
    <= n_tiles. Pad slots gather row 0 and scatter to junk rows.
    """
    n = len(g)
    if n > n_tiles * T or (n and np.bincount(s).max() > n_tiles):
        return None
    ga = np.zeros((n_tiles, T), np.int64)
    sa = np.empty((n_tiles, T), np.int64)
    sa[:] = junk[None, :]
    p = np.arange(n)
    ga[p % n_tiles, p // n_tiles] = g
    sa[p % n_tiles, p // n_tiles] = s
    return ga, sa


def _route(vertex, edges):
    """Per-core idx tensors for both gather/scatter stages, or None if the
    (astronomically unlikely) static capacities are exceeded."""
    le = edges % EDGE_SH
    xe_reg = le // EDGE_REG                    # region within shard
    xe_shard_row = EDGE_REG_P * xe_reg + (le - EDGE_REG * xe_reg)
    xe_tbl_row = EDGE_REG_P * (edges // EDGE_SH) + (le - EDGE_REG * xe_reg)
    owner_a = edges // EDGE_SH
    chunk_a = vertex // A_CHUNK_ROWS
    owner_c = vertex // NODE_SH

    ar = np.arange(T)
    junk_a = EDGE_REG_P * (ar % 2) + EDGE_REG + (ar // 2) % (EDGE_REG_P - EDGE_REG)
    junk_c = NODE_SH + ar % (NODE_SH_P - NODE_SH)

    ga_all, sa_all, gc_all, sc_all = [], [], [], []
    for m in range(NC):
        ga_m = np.zeros((A_TILES, T), np.int64)
        sa_m = np.empty((A_TILES, T), np.int64)
        sa_m[:] = junk_a[None, :]
        for c in range(A_CHUNKS):
            sel = np.nonzero((owner_a == m) & (chunk_a == c))[0]
            s = xe_shard_row[sel]
            order = np.argsort(s, kind="stable")
            d = _deal(vertex[sel][order] - A_CHUNK_ROWS * c, s[order], A_TPC,
                      junk_a)
            if d is None:
                return None
            ga_m[c * A_TPC:(c + 1) * A_TPC] = d[0]
            sa_m[c * A_TPC:(c + 1) * A_TPC] = d[1]
        gc_m = np.zeros((C_TILES, T), np.int64)
        sc_m = np.empty((C_TILES, T), np.int64)
        sc_m[:] = junk_c[None, :]
        for r in range(C_SEGS):
            sel = np.nonzero((owner_c == m) & (xe_reg == r))[0]
            s = vertex[sel] - NODE_SH * m
            order = np.argsort(s, kind="stable")
            d = _deal(xe_tbl_row[sel][order], s[order], C_TPC, junk_c)
            if d is None:
                return None
            gc_m[r * C_TPC:(r + 1) * C_TPC] = d[0]
            sc_m[r * C_TPC:(r + 1) * C_TPC] = d[1]
        ga_all.append(_wrap16(ga_m))
        sa_all.append(_wrap16(sa_m))
        gc_all.append(_wrap16(gc_m))
        sc_all.append(_wrap16(sc_m))
    return ga_all, sa_all, gc_all, sc_all


def _numpy_fallback(X, vertex, edges, w_b, w_a, w_c, b_b, b_a, b_c):
    Xe = np.zeros((E, F), np.float32)
    np.add.at(Xe, edges, X[vertex])
    Xv2 = np.zeros((N, F), np.float32)
    np.add.at(Xv2, vertex, Xe[edges])
    deg = np.bincount(vertex, minlength=N).astype(np.float32)[:, None]
    Xv = np.concatenate([deg * X, Xv2], axis=1)
    center = Xv @ w_b + b_b
    aXv = np.abs(Xv)
    return (center.astype(np.float32),
            (center - (aXv @ w_a + b_a)).astype(np.float32),
            (center + (aXv @ w_c + b_c)).astype(np.float32))


# ------------------------------------------------------------- bass program
def _build_program():
    from concourse import bacc, tile
    import concourse.mybir as mybir

    f32 = mybir.dt.float32
    mmdt = {"f32": f32, "f32r": mybir.dt.float32r,
            "bf16": mybir.dt.bfloat16}[MM_DT]
    bdt = f32 if MM_DT == "f32r" else mmdt
    i16 = mybir.dt.int16

    nc = bacc.Bacc(None, target_bir_lowering=False, debug=False,
                   num_devices=NC, num_swdge_queues=4)

    xfull = nc.dram_tensor("xfull", [N, F], f32, kind="ExternalInput")
    xshard = nc.dram_tensor("xshard", [NODE_SH, F], f32, kind="ExternalInput")
    ga = nc.dram_tensor("ga", [A_TILES, 128, T // 16], i16, kind="ExternalInput")
    sa = nc.dram_tensor("sa", [A_TILES, 128, T // 16], i16, kind="ExternalInput")
    gc = nc.dram_tensor("gc", [C_TILES, 128, T // 16], i16, kind="ExternalInput")
    sc = nc.dram_tensor("sc", [C_TILES, 128, T // 16], i16, kind="ExternalInput")
    deg = nc.dram_tensor("deg", [ND_TILES, 128, 1], f32, kind="ExternalInput")
    wts_d = {nm: nc.dram_tensor(nm, [F, F], mmdt, kind="ExternalInput")
             for nm in ("wb1", "wb2", "wa1n", "wa2n", "wc1", "wc2")}
    bias_d = {nm: nc.dram_tensor(nm, [1, F], bdt, kind="ExternalInput")
              for nm in ("bias_c", "bias_l", "bias_r")}
    outs = {nm: nc.dram_tensor(nm, [NODE_SH, F], f32, kind="ExternalOutput")
            for nm in ("center", "hl", "hr")}

    xe_acc = [nc.dram_tensor(f"xe_acc{k}", [EDGE_SH_P, F], f32)
              for k in range(N_ACC)]
    xe_sum = nc.dram_tensor("xe_sum", [EDGE_SH_P, F], f32)
    xe_tbl = [nc.dram_tensor(f"xe_tbl{r}", [XE_TBL, F], f32)
              for r in range(2)]
    xv2_acc = [nc.dram_tensor(f"xv2_acc{k}", [NODE_SH_P, F], f32)
               for k in range(N_ACC)]

    eye_d = nc.inline_tensor(np.eye(128, dtype=np.float32), name="eye128")

    def flat(t, lo=None, hi=None):
        ap = t.ap() if lo is None else t[lo:hi, :]
        return ap.rearrange("(p a) f -> p (a f)", p=128)

    with tile.TileContext(nc) as tc:
        with (
            tc.tile_pool(name="cpool", bufs=1) as cpool,
            tc.tile_pool(name="ipool", bufs=8) as ipool,
            tc.tile_pool(name="dpool", bufs=8) as dpool,
            tc.tile_pool(name="spool", bufs=3) as spool,
            tc.tile_pool(name="tpool", bufs=2) as tpool,
            tc.tile_pool(name="opool", bufs=2) as opool,
            tc.tile_pool(name="ps_tr", bufs=1, space="PSUM") as ps_tr,
            tc.tile_pool(name="ps_mm", bufs=2, space="PSUM") as ps_mm,
        ):
            # constants
            zero = cpool.tile([128, 2048], f32)
            nc.vector.memset(zero[:], 0.0)
            ident = cpool.tile([128, 128], f32)
            nc.sync.dma_start(ident[:], eye_d[:])
            ones = cpool.tile([1, F], bdt)
            nc.vector.memset(ones[:], 1.0)
            wts = {}
            for nm, d in wts_d.items():
                wtile = cpool.tile([F, F], mmdt, tag=nm)
                nc.sync.dma_start(wtile[:], d[:])
                wts[nm] = wtile
            biases = {}
            bias_bc = {}
            for nm, d in bias_d.items():
                btile = cpool.tile([1, F], bdt, tag=nm)
                nc.sync.dma_start(btile[:], d[:])
                biases[nm] = btile
            for nm in bias_d:
                bps = ps_mm.tile([128, F], f32, tag="hsum_ps")
                nc.tensor.matmul(bps[:], ones[:], biases[nm][:],
                                 start=True, stop=True)
                bct = cpool.tile([128, F], f32, tag=f"bc_{nm}")
                nc.vector.tensor_copy(bct[:], bps[:])
                bias_bc[nm] = bct

            # zero the DRAM accumulators
            if "Z" in STAGES:
                for t in xe_acc + xv2_acc:
                    fl = flat(t)
                    for off in range(0, fl.shape[1], 2048):
                        sz = min(2048, fl.shape[1] - off)
                        nc.sync.dma_start(fl[:, off:off + sz], zero[:, :sz])

            # stage A: X[vertex] scatter-added by edge
            for ti in range(A_TILES if "A" in STAGES else 0):
                c = ti // A_TPC
                gt = ipool.tile([128, T // 16], i16, tag="gt")
                st = ipool.tile([128, T // 16], i16, tag="st")
                nc.sync.dma_start(gt[:], ga[ti])
                nc.sync.dma_start(st[:], sa[ti])
                dat = dpool.tile([128, T // 128, F], f32, tag="dat")
                nc.gpsimd.dma_gather(
                    dat[:], xfull[c * A_CHUNK_ROWS:(c + 1) * A_CHUNK_ROWS, :],
                    gt[:], T, T, F, queue_num=ti % 4)
                nc.gpsimd.dma_scatter_add(
                    xe_acc[ti % N_ACC][:], dat[:], st[:], T, T, F,
                    queue_num=(ti + 2) % 4)

            # stage A.5 + B: per edge region, sum the accumulators and
            # all-gather that region's shard (region 0 lands first so
            # stage C region-0 work starts early)
            if "A" in STAGES:
                for r in range(2):
                    lo, hi = r * EDGE_REG_P, (r + 1) * EDGE_REG_P
                    fls = [flat(t, lo, hi) for t in xe_acc]
                    fs = flat(xe_sum, lo, hi)
                    w_tot = fs.shape[1]
                    for off in range(0, w_tot, 1600):
                        sz = min(1600, w_tot - off)
                        ta = spool.tile([128, 1600], f32, tag="sum_a")
                        tb = spool.tile([128, 1600], f32, tag="sum_b")
                        nc.sync.dma_start(ta[:, :sz], fls[0][:, off:off + sz])
                        nc.scalar.dma_start(tb[:, :sz], fls[1][:, off:off + sz])
                        nc.vector.tensor_add(ta[:, :sz], ta[:, :sz], tb[:, :sz])
                        nc.sync.dma_start(fs[:, off:off + sz], ta[:, :sz])
                    if "B" in STAGES:
                        nc.gpsimd.collective_compute(
                            "AllGather", mybir.AluOpType.bypass,
                            replica_groups=[list(range(NC))],
                            ins=[xe_sum[lo:hi, :].opt()],
                            outs=[xe_tbl[r].ap().opt()],
                        )

            # stage C: Xe[edges] scatter-added by vertex
            for ti in range(C_TILES if "C" in STAGES else 0):
                r = ti // C_TPC
                gt = ipool.tile([128, T // 16], i16, tag="gt")
                st = ipool.tile([128, T // 16], i16, tag="st")
                nc.sync.dma_start(gt[:], gc[ti])
                nc.sync.dma_start(st[:], sc[ti])
                dat = dpool.tile([128, T // 128, F], f32, tag="dat")
                nc.gpsimd.dma_gather(dat[:], xe_tbl[r][:], gt[:], T, T, F,
                                     queue_num=ti % 4)
                nc.gpsimd.dma_scatter_add(
                    xv2_acc[ti % N_ACC][:], dat[:], st[:], T, T, F,
                    queue_num=(ti + 2) % 4)

            # stage D: dense head
            Abs = mybir.ActivationFunctionType.Abs
            Copy = mybir.ActivationFunctionType.Copy
            for nt in range(ND_TILES if "D" in STAGES else 0):
                rows = min(128, NODE_SH - nt * 128)
                r0 = nt * 128
                xt = spool.tile([128, F], f32, tag="xt")
                nc.sync.dma_start(xt[:rows, :], xshard[r0:r0 + rows, :])
                dg = spool.tile([128, 1], f32, tag="dg")
                nc.sync.dma_start(dg[:], deg[nt])
                va = spool.tile([128, F], f32, tag="va")
                vb = spool.tile([128, F], f32, tag="vb")
                nc.scalar.dma_start(va[:], xv2_acc[0][r0:r0 + 128, :])
                nc.sync.dma_start(vb[:], xv2_acc[1][r0:r0 + 128, :])
                nc.vector.tensor_add(va[:], va[:], vb[:])

                h1 = spool.tile([128, F], f32, tag="h1")
                nc.vector.tensor_scalar_mul(h1[:], xt[:], dg[:, 0:1])

                h1T_ps = ps_tr.tile([128, F], f32, tag="h1T_ps")
                nc.tensor.transpose(h1T_ps[:], h1[:], ident[:])
                h2T_ps = ps_tr.tile([128, F], f32, tag="h2T_ps")
                nc.tensor.transpose(h2T_ps[:], va[:], ident[:])

                h1T = tpool.tile([128, F], mmdt, tag="h1T")
                nc.vector.tensor_copy(h1T[:], h1T_ps[:])
                h2T = tpool.tile([128, F], mmdt, tag="h2T")
                nc.vector.tensor_copy(h2T[:], h2T_ps[:])
                a1T = tpool.tile([128, F], mmdt, tag="a1T")
                nc.scalar.activation(a1T[:], h1T_ps[:], Abs)
                a2T = tpool.tile([128, F], mmdt, tag="a2T")
                nc.scalar.activation(a2T[:], h2T_ps[:], Abs)

                groups = (
                    ("hsum_ps", (("h1T", "wb1"), ("h2T", "wb2"))),
                    ("lpart_ps", (("a1T", "wa1n"), ("a2T", "wa2n"))),
                    ("rpart_ps", (("a1T", "wc1"), ("a2T", "wc2"))),
                )
                lhs = {"h1T": h1T, "h2T": h2T, "a1T": a1T, "a2T": a2T}
                ps_out = {}
                for psname, terms in groups:
                    ps = ps_mm.tile([128, F], f32, tag=psname)
                    for i, (ln, wn) in enumerate(terms):
                        nc.tensor.matmul(ps[:], lhs[ln][:], wts[wn][:],
                                         start=(i == 0), stop=(i == len(terms) - 1))
                    ps_out[psname] = ps
                ot_c = opool.tile([128, F], f32, tag="o_center")
                nc.vector.tensor_add(ot_c[:], bias_bc["bias_c"][:],
                                     ps_out["hsum_ps"][:])
                nc.sync.dma_start(outs["center"][r0:r0 + rows, :], ot_c[:rows, :])
                for part, bias, oname in (("lpart_ps", "bias_l", "hl"),
                                          ("rpart_ps", "bias_r", "hr")):
                    tmp = opool.tile([128, F], f32, tag=f"t_{oname}")
                    nc.vector.tensor_add(tmp[:], bias_bc[bias][:],
                                         ps_out[part][:])
                    ot = opool.tile([128, F], f32, tag=f"o_{oname}")
                    nc.vector.tensor_add(ot[:], tmp[:], ps_out["hsum_ps"][:])
                    nc.sync.dma_start(outs[oname][r0:r0 + rows, :], ot[:rows, :])

    nc.compile()
    return nc


# ------------------------------------------------------------------- driver
def kernel(X, vertex, edges, X0, n_edges, w_b, w_a, w_c, b_b, b_a, b_c):
    from concourse.bass_utils import run_bass_kernel_spmd
    import ml_dtypes

    X = np.ascontiguousarray(np.asarray(X, dtype=np.float32))
    vertex = np.asarray(vertex).astype(np.int64)
    edges = np.asarray(edges).astype(np.int64)
    w_b = np.asarray(w_b, dtype=np.float32)
    w_a = np.asarray(w_a, dtype=np.float32)
    w_c = np.asarray(w_c, dtype=np.float32)
    b_b = np.asarray(b_b, dtype=np.float32).reshape(1, F)
    b_a = np.asarray(b_a, dtype=np.float32).reshape(1, F)
    b_c = np.asarray(b_c, dtype=np.float32).reshape(1, F)

    routed = _route(vertex, edges)
    if routed is None:
        return _numpy_fallback(X, vertex, edges, w_b, w_a, w_c, b_b, b_a, b_c)
    ga_all, sa_all, gc_all, sc_all = routed

    if "nc" not in _STATE:
        _STATE["nc"] = _build_program()
    nc = _STATE["nc"]

    npmm = ml_dtypes.bfloat16 if MM_DT == "bf16" else np.float32
    deg_full = np.bincount(vertex, minlength=N).astype(np.float32)
    wmats = {
        "wb1": w_b[:F], "wb2": w_b[F:],
        "wa1n": -w_a[:F], "wa2n": -w_a[F:],
        "wc1": w_c[:F], "wc2": w_c[F:],
    }
    bmats = {"bias_c": b_b, "bias_l": b_b - b_a, "bias_r": b_b + b_c}

    in_maps = []
    for m in range(NC):
        dshard = np.zeros(NODE_SH_P, np.float32)
        dshard[:NODE_SH] = deg_full[m * NODE_SH:(m + 1) * NODE_SH]
        im = {
            "xfull": X,
            "xshard": np.ascontiguousarray(X[m * NODE_SH:(m + 1) * NODE_SH]),
            "ga": ga_all[m], "sa": sa_all[m],
            "gc": gc_all[m], "sc": sc_all[m],
            "deg": dshard.reshape(ND_TILES, 128, 1),
        }
        for nm, w in wmats.items():
            im[nm] = np.ascontiguousarray(w.astype(npmm))
        npb = np.float32 if MM_DT == "f32r" else npmm
        for nm, b in bmats.items():
            im[nm] = np.ascontiguousarray(b.astype(npb))
        in_maps.append(im)

    res = run_bass_kernel_spmd(nc, in_maps, list(range(NC)))
    center = np.concatenate([res.results[m]["center"] for m in range(NC)])
    hl = np.concatenate([res.results[m]["hl"] for m in range(NC)])
    hr = np.concatenate([res.results[m]["hr"] for m in range(NC)])
    return center, hl, hr


# revision 11
# speedup vs baseline: 1.1585x; 1.0817x over previous
"""Trainium2 Bass kernel for nn_CrispToFuzzyConv (hypergraph message passing).

Math (see reference):
  Xe   = segment_sum(X[vertex], edges, E)                 # round 1
  Xv   = segment_sum(concat([X[vertex], Xe[edges]]), vertex, N)
       = concat([deg * X, Xv2]),  Xv2 = segment_sum(Xe[edges], vertex, N)
  center = Xv @ w_b + b_b
  HL = center - (|Xv| @ w_a + b_a)
  HR = center + (|Xv| @ w_c + b_c)

Distribution over 8 NeuronCores:
  - round 1 sharded by edge owner: dma_gather rows from replicated X,
    dma_scatter_add into 4 round-robin per-core Xe accumulators (4
    independent WAW chains on 4 SWDGE queues), summed into the padded
    Xe shard
  - 2 AllGather collectives (one per edge region) -> two 25600-row Xe
    tables; region-0 stage-C work starts while region-1 is still landing
  - round 2 sharded by vertex owner: gather from the Xe tables + scatter
    into 4 Xv2 accumulators
  - dense stage per core: deg-scaling (ACT per-partition scale), PE
    transposes, 13 matmuls/tile with bias folded in as K=1 ones x bias

Key hardware constraints baked in (established empirically):
  - dma_gather/dma_scatter_add indices are int16 -> gather tables are
    chunked to <= 32768 rows; <= 1024 indices per call (the SWDGE ring
    rejects 1280+)
  - duplicate scatter rows WITHIN one call race (lost updates) -> tokens
    are dealt round-robin over a segment's tiles so each call's rows are
    unique; pad slots gather row 0 and scatter garbage to junk rows
  - consecutive scatter calls to one tensor serialize (Tile WAW) and
    accumulate exactly -> 4 alternating accumulators give 4 parallel
    chains
  - collective in/out tensors must be Internal, addr_space Local (Shared
    breaks dma_gather reading the output)
  - num_swdge_queues=4 parallelizes Q7 descriptor generation ~3x
"""

import os
import numpy as np

# ---------------------------------------------------------------- constants
N = 100000
E = 50000
NNZ = 300000
F = 128
NC = 8

NODE_SH = N // NC            # 12500
EDGE_SH = E // NC            # 6250
NODE_SH_P = 12544            # 98 * 128 (rows >= 12500 are junk)
EDGE_REG = EDGE_SH // 2      # 3125 real edges per region
EDGE_REG_P = 3200            # 25 * 128 (rows >= 3125 are junk)
EDGE_SH_P = 2 * EDGE_REG_P   # 6400 (regions stacked)
XE_TBL = NC * EDGE_REG_P     # 25600 rows per region table (int16-safe)

T = 1024                     # tokens per gather/scatter call (hard max)
A_CHUNKS = 4                 # X gather table chunks (25000 rows)
A_CHUNK_ROWS = N // A_CHUNKS
A_TPC = 10                   # stage-A tiles per (core, chunk) segment
C_SEGS = 2                   # stage-C segments per core (one per region table)
C_TPC = 20                   # stage-C tiles per segment
A_TILES = A_CHUNKS * A_TPC   # 40
C_TILES = C_SEGS * C_TPC     # 40
ND_TILES = NODE_SH_P // 128  # 98
N_ACC = 2                    # parallel scatter chains per accumulator

MM_DT = os.environ.get("BASS_GNN_MM_DT", "f32r")  # f32 | f32r | bf16
STAGES = os.environ.get("BASS_GNN_STAGES", "ZABCD")

_STATE = {}


# ---------------------------------------------------------------- host side
def _wrap16(idx):
    """[n_tiles, T] int -> [n_tiles, 128, T//16] int16 (idx i at partition
    i%16, col i//16; replicated across the 8 groups of 16 partitions)."""
    n_tiles = idx.shape[0]
    t = idx.reshape(n_tiles, T // 16, 16).transpose(0, 2, 1).astype(np.int16)
    return np.ascontiguousarray(np.tile(t, (1, 8, 1)))


def _deal(g, s, n_tiles, junk):
    """Pack one segment's tokens into n_tiles gather/scatter idx tiles.

    g, s: per-token gather idx / scatter row, s sorted ascending.
    Round-robin dealing (token p -> tile p % n_tiles, slot p // n_tiles)
    keeps each tile's scatter rows unique when every row's multiplicity
    <= n_tiles. Pad slots gather row 0 and scatter to junk rows.
    """
    n = len(g)
    if n > n_tiles * T or (n and np.bincount(s).max() > n_tiles):
        return None
    ga = np.zeros((n_tiles, T), np.int64)
    sa = np.empty((n_tiles, T), np.int64)
    sa[:] = junk[None, :]
    p = np.arange(n)
    ga[p % n_tiles, p // n_tiles] = g
    sa[p % n_tiles, p // n_tiles] = s
    return ga, sa


def _route(vertex, edges):
    """Per-core idx tensors for both gather/scatter stages, or None if the
    (astronomically unlikely) static capacities are exceeded."""
    le = edges % EDGE_SH
    xe_reg = le // EDGE_REG                    # region within shard
    xe_shard_row = EDGE_REG_P * xe_reg + (le - EDGE_REG * xe_reg)
    xe_tbl_row = EDGE_REG_P * (edges // EDGE_SH) + (le - EDGE_REG * xe_reg)
    owner_a = edges // EDGE_SH
    chunk_a = vertex // A_CHUNK_ROWS
    owner_c = vertex // NODE_SH

    ar = np.arange(T)
    junk_a = EDGE_REG_P * (ar % 2) + EDGE_REG + (ar // 2) % (EDGE_REG_P - EDGE_REG)
    junk_c = NODE_SH + ar % (NODE_SH_P - NODE_SH)

    ga_all, sa_all, gc_all, sc_all = [], [], [], []
    for m in range(NC):
        ga_m = np.zeros((A_TILES, T), np.int64)
        sa_m = np.empty((A_TILES, T), np.int64)
        sa_m[:] = junk_a[None, :]
        for c in range(A_CHUNKS):
            sel = np.nonzero((owner_a == m) & (chunk_a == c))[0]
            s = xe_shard_row[sel]
            order = np.argsort(s, kind="stable")
            d = _deal(vertex[sel][order] - A_CHUNK_ROWS * c, s[order], A_TPC,
                      junk_a)
            if d is None:
                return None
            ga_m[c * A_TPC:(c + 1) * A_TPC] = d[0]
            sa_m[c * A_TPC:(c + 1) * A_TPC] = d[1]
        gc_m = np.zeros((C_TILES, T), np.int64)
        sc_m = np.empty((C_TILES, T), np.int64)
        sc_m[:] = junk_c[None, :]
        for r in range(C_SEGS):
            sel = np.nonzero((owner_c == m) & (xe_reg == r))[0]
            s = vertex[sel] - NODE_SH * m
            order = np.argsort(s, kind="stable")
            d = _deal(xe_tbl_row[sel][order], s[order], C_TPC, junk_c)
            if d is None:
                return None
            gc_m[r * C_TPC:(r + 1) * C_TPC] = d[0]
            sc_m[r * C_TPC:(r + 1) * C_TPC] = d[1]
        ga_all.append(_wrap16(ga_m))
        sa_all.append(_wrap16(sa_m))
        gc_all.append(_wrap16(gc_m))
        sc_all.append(_wrap16(sc_m))
    return ga_all, sa_all, gc_all, sc_all


def _numpy_fallback(X, vertex, edges, w_b, w_a, w_c, b_b, b_a, b_c):
    Xe = np.zeros((E, F), np.float32)
    np.add.at(Xe, edges, X[vertex])
    Xv2 = np.zeros((N, F), np.float32)
    np.add.at(Xv2, vertex, Xe[edges])
    deg = np.bincount(vertex, minlength=N).astype(np.float32)[:, None]
    Xv = np.concatenate([deg * X, Xv2], axis=1)
    center = Xv @ w_b + b_b
    aXv = np.abs(Xv)
    return (center.astype(np.float32),
            (center - (aXv @ w_a + b_a)).astype(np.float32),
            (center + (aXv @ w_c + b_c)).astype(np.float32))


# ------------------------------------------------------------- bass program
def _build_program():
    from concourse import bacc, tile
    import concourse.mybir as mybir

    f32 = mybir.dt.float32
    mmdt = {"f32": f32, "f32r": mybir.dt.float32r,
            "bf16": mybir.dt.bfloat16}[MM_DT]
    bdt = f32 if MM_DT == "f32r" else mmdt
    i16 = mybir.dt.int16

    nc = bacc.Bacc(None, target_bir_lowering=False, debug=False,
                   num_devices=NC, num_swdge_queues=4)

    xfull = nc.dram_tensor("xfull", [N, F], f32, kind="ExternalInput")
    xshard = nc.dram_tensor("xshard", [NODE_SH, F], f32, kind="ExternalInput")
    ga = nc.dram_tensor("ga", [A_TILES, 128, T // 16], i16, kind="ExternalInput")
    sa = nc.dram_tensor("sa", [A_TILES, 128, T // 16], i16, kind="ExternalInput")
    gc = nc.dram_tensor("gc", [C_TILES, 128, T // 16], i16, kind="ExternalInput")
    sc = nc.dram_tensor("sc", [C_TILES, 128, T // 16], i16, kind="ExternalInput")
    deg = nc.dram_tensor("deg", [128, ND_TILES], f32, kind="ExternalInput")
    wts_d = {nm: nc.dram_tensor(nm, [F, F], mmdt, kind="ExternalInput")
             for nm in ("wb1", "wb2", "wa1n", "wa2n", "wc1", "wc2")}
    bias_d = {nm: nc.dram_tensor(nm, [1, F], bdt, kind="ExternalInput")
              for nm in ("bias_c", "bias_l", "bias_r")}
    out3 = nc.dram_tensor("out3", [NODE_SH, 3 * F], f32, kind="ExternalOutput")

    xe_acc = [nc.dram_tensor(f"xe_acc{k}", [EDGE_SH_P, F], f32)
              for k in range(N_ACC)]
    xe_sum = nc.dram_tensor("xe_sum", [EDGE_SH_P, F], f32)
    xe_tbl = [nc.dram_tensor(f"xe_tbl{r}", [XE_TBL, F], f32)
              for r in range(2)]
    xv2_acc = [nc.dram_tensor(f"xv2_acc{k}", [NODE_SH_P, F], f32)
               for k in range(N_ACC)]

    eye_d = nc.inline_tensor(np.eye(128, dtype=np.float32), name="eye128")

    def flat(t, lo=None, hi=None):
        ap = t.ap() if lo is None else t[lo:hi, :]
        return ap.rearrange("(p a) f -> p (a f)", p=128)

    with tile.TileContext(nc) as tc:
        with (
            tc.tile_pool(name="cpool", bufs=1) as cpool,
            tc.tile_pool(name="ipool", bufs=8) as ipool,
            tc.tile_pool(name="dpool", bufs=8) as dpool,
            tc.tile_pool(name="spool", bufs=3) as spool,
            tc.tile_pool(name="tpool", bufs=2) as tpool,
            tc.tile_pool(name="opool", bufs=2) as opool,
            tc.tile_pool(name="ps_tr", bufs=1, space="PSUM") as ps_tr,
            tc.tile_pool(name="ps_mm", bufs=2, space="PSUM") as ps_mm,
        ):
            # constants
            zero = cpool.tile([128, 2048], f32)
            nc.vector.memset(zero[:], 0.0)
            ident = cpool.tile([128, 128], f32)
            nc.sync.dma_start(ident[:], eye_d[:])
            deg_all = cpool.tile([128, ND_TILES], f32)
            nc.sync.dma_start(deg_all[:], deg[:])
            ones = cpool.tile([1, F], bdt)
            nc.vector.memset(ones[:], 1.0)
            wts = {}
            for nm, d in wts_d.items():
                wtile = cpool.tile([F, F], mmdt, tag=nm)
                nc.sync.dma_start(wtile[:], d[:])
                wts[nm] = wtile
            biases = {}
            bias_bc = {}
            for nm, d in bias_d.items():
                btile = cpool.tile([1, F], bdt, tag=nm)
                nc.sync.dma_start(btile[:], d[:])
                biases[nm] = btile
            for nm in bias_d:
                bps = ps_mm.tile([128, F], f32, tag="hsum_ps")
                nc.tensor.matmul(bps[:], ones[:], biases[nm][:],
                                 start=True, stop=True)
                bct = cpool.tile([128, F], f32, tag=f"bc_{nm}")
                nc.vector.tensor_copy(bct[:], bps[:])
                bias_bc[nm] = bct

            # zero the DRAM accumulators
            if "Z" in STAGES:
                for t in xe_acc + xv2_acc:
                    fl = flat(t)
                    for off in range(0, fl.shape[1], 2048):
                        sz = min(2048, fl.shape[1] - off)
                        nc.sync.dma_start(fl[:, off:off + sz], zero[:, :sz])

            # stage A: X[vertex] scatter-added by edge
            for ti in range(A_TILES if "A" in STAGES else 0):
                c = ti // A_TPC
                gt = ipool.tile([128, T // 16], i16, tag="gt")
                st = ipool.tile([128, T // 16], i16, tag="st")
                nc.sync.dma_start(gt[:], ga[ti])
                nc.sync.dma_start(st[:], sa[ti])
                dat = dpool.tile([128, T // 128, F], f32, tag="dat")
                nc.gpsimd.dma_gather(
                    dat[:], xfull[c * A_CHUNK_ROWS:(c + 1) * A_CHUNK_ROWS, :],
                    gt[:], T, T, F, queue_num=ti % 4)
                nc.gpsimd.dma_scatter_add(
                    xe_acc[ti % N_ACC][:], dat[:], st[:], T, T, F,
                    queue_num=(ti + 2) % 4)

            # stage A.5 + B: per edge region, sum the accumulators and
            # all-gather that region's shard (region 0 lands first so
            # stage C region-0 work starts early)
            if "A" in STAGES:
                for r in range(2):
                    lo, hi = r * EDGE_REG_P, (r + 1) * EDGE_REG_P
                    fls = [flat(t, lo, hi) for t in xe_acc]
                    fs = flat(xe_sum, lo, hi)
                    w_tot = fs.shape[1]
                    for off in range(0, w_tot, 1600):
                        sz = min(1600, w_tot - off)
                        ta = spool.tile([128, 1600], f32, tag="sum_a")
                        tb = spool.tile([128, 1600], f32, tag="sum_b")
                        nc.sync.dma_start(ta[:, :sz], fls[0][:, off:off + sz])
                        nc.scalar.dma_start(tb[:, :sz], fls[1][:, off:off + sz])
                        nc.vector.tensor_add(ta[:, :sz], ta[:, :sz], tb[:, :sz])
                        nc.sync.dma_start(fs[:, off:off + sz], ta[:, :sz])
                    if "B" in STAGES:
                        nc.gpsimd.collective_compute(
                            "AllGather", mybir.AluOpType.bypass,
                            replica_groups=[list(range(NC))],
                            ins=[xe_sum[lo:hi, :].opt()],
                            outs=[xe_tbl[r].ap().opt()],
                        )

            # stage C: Xe[edges] scatter-added by vertex
            for ti in range(C_TILES if "C" in STAGES else 0):
                r = ti // C_TPC
                gt = ipool.tile([128, T // 16], i16, tag="gt")
                st = ipool.tile([128, T // 16], i16, tag="st")
                nc.sync.dma_start(gt[:], gc[ti])
                nc.sync.dma_start(st[:], sc[ti])
                dat = dpool.tile([128, T // 128, F], f32, tag="dat")
                nc.gpsimd.dma_gather(dat[:], xe_tbl[r][:], gt[:], T, T, F,
                                     queue_num=ti % 4)
                nc.gpsimd.dma_scatter_add(
                    xv2_acc[ti % N_ACC][:], dat[:], st[:], T, T, F,
                    queue_num=(ti + 2) % 4)

            # stage D: dense head
            Abs = mybir.ActivationFunctionType.Abs
            Copy = mybir.ActivationFunctionType.Copy
            for nt in range(ND_TILES if "D" in STAGES else 0):
                rows = min(128, NODE_SH - nt * 128)
                r0 = nt * 128
                xt = spool.tile([128, F], f32, tag="xt")
                nc.sync.dma_start(xt[:rows, :], xshard[r0:r0 + rows, :])
                va = spool.tile([128, F], f32, tag="va")
                vb = spool.tile([128, F], f32, tag="vb")
                nc.scalar.dma_start(va[:], xv2_acc[0][r0:r0 + 128, :])
                nc.sync.dma_start(vb[:], xv2_acc[1][r0:r0 + 128, :])
                nc.vector.tensor_add(va[:], va[:], vb[:])

                h1 = spool.tile([128, F], f32, tag="h1")
                nc.vector.tensor_scalar_mul(h1[:], xt[:], deg_all[:, nt:nt + 1])

                h1T_ps = ps_tr.tile([128, F], f32, tag="h1T_ps")
                nc.tensor.transpose(h1T_ps[:], h1[:], ident[:])
                h2T_ps = ps_tr.tile([128, F], f32, tag="h2T_ps")
                nc.tensor.transpose(h2T_ps[:], va[:], ident[:])

                h1T = tpool.tile([128, F], mmdt, tag="h1T")
                nc.vector.tensor_copy(h1T[:], h1T_ps[:])
                h2T = tpool.tile([128, F], mmdt, tag="h2T")
                nc.vector.tensor_copy(h2T[:], h2T_ps[:])
                a1T = tpool.tile([128, F], mmdt, tag="a1T")
                nc.scalar.activation(a1T[:], h1T_ps[:], Abs)
                a2T = tpool.tile([128, F], mmdt, tag="a2T")
                nc.scalar.activation(a2T[:], h2T_ps[:], Abs)

                groups = (
                    ("hsum_ps", (("h1T", "wb1"), ("h2T", "wb2"))),
                    ("lpart_ps", (("a1T", "wa1n"), ("a2T", "wa2n"))),
                    ("rpart_ps", (("a1T", "wc1"), ("a2T", "wc2"))),
                )
                lhs = {"h1T": h1T, "h2T": h2T, "a1T": a1T, "a2T": a2T}
                ps_out = {}
                for psname, terms in groups:
                    ps = ps_mm.tile([128, F], f32, tag=psname)
                    for i, (ln, wn) in enumerate(terms):
                        nc.tensor.matmul(ps[:], lhs[ln][:], wts[wn][:],
                                         start=(i == 0), stop=(i == len(terms) - 1))
                    ps_out[psname] = ps
                ot = opool.tile([128, 3 * F], f32, tag="ot")
                nc.vector.tensor_add(ot[:, 0:F], bias_bc["bias_c"][:],
                                     ps_out["hsum_ps"][:])
                for k, (part, bias) in enumerate((("lpart_ps", "bias_l"),
                                                  ("rpart_ps", "bias_r"))):
                    tmp = opool.tile([128, F], f32, tag=f"t{k}")
                    nc.vector.tensor_add(tmp[:], bias_bc[bias][:],
                                         ps_out[part][:])
                    nc.vector.tensor_add(ot[:, (k + 1) * F:(k + 2) * F],
                                         tmp[:], ps_out["hsum_ps"][:])
                nc.sync.dma_start(out3[r0:r0 + rows, :], ot[:rows, :])

    nc.compile()
    return nc


# ------------------------------------------------------------------- driver
def kernel(X, vertex, edges, X0, n_edges, w_b, w_a, w_c, b_b, b_a, b_c):
    from concourse.bass_utils import run_bass_kernel_spmd
    import ml_dtypes

    X = np.ascontiguousarray(np.asarray(X, dtype=np.float32))
    vertex = np.asarray(vertex).astype(np.int64)
    edges = np.asarray(edges).astype(np.int64)
    w_b = np.asarray(w_b, dtype=np.float32)
    w_a = np.asarray(w_a, dtype=np.float32)
    w_c = np.asarray(w_c, dtype=np.float32)
    b_b = np.asarray(b_b, dtype=np.float32).reshape(1, F)
    b_a = np.asarray(b_a, dtype=np.float32).reshape(1, F)
    b_c = np.asarray(b_c, dtype=np.float32).reshape(1, F)

    routed = _route(vertex, edges)
    if routed is None:
        return _numpy_fallback(X, vertex, edges, w_b, w_a, w_c, b_b, b_a, b_c)
    ga_all, sa_all, gc_all, sc_all = routed

    if "nc" not in _STATE:
        _STATE["nc"] = _build_program()
    nc = _STATE["nc"]

    npmm = ml_dtypes.bfloat16 if MM_DT == "bf16" else np.float32
    deg_full = np.bincount(vertex, minlength=N).astype(np.float32)
    wmats = {
        "wb1": w_b[:F], "wb2": w_b[F:],
        "wa1n": -w_a[:F], "wa2n": -w_a[F:],
        "wc1": w_c[:F], "wc2": w_c[F:],
    }
    bmats = {"bias_c": b_b, "bias_l": b_b - b_a, "bias_r": b_b + b_c}

    in_maps = []
    for m in range(NC):
        dshard = np.zeros(NODE_SH_P, np.float32)
        dshard[:NODE_SH] = deg_full[m * NODE_SH:(m + 1) * NODE_SH]
        im = {
            "xfull": X,
            "xshard": np.ascontiguousarray(X[m * NODE_SH:(m + 1) * NODE_SH]),
            "ga": ga_all[m], "sa": sa_all[m],
            "gc": gc_all[m], "sc": sc_all[m],
            "deg": np.ascontiguousarray(dshard.reshape(ND_TILES, 128).T),
        }
        for nm, w in wmats.items():
            im[nm] = np.ascontiguousarray(w.astype(npmm))
        npb = np.float32 if MM_DT == "f32r" else npmm
        for nm, b in bmats.items():
            im[nm] = np.ascontiguousarray(b.astype(npb))
        in_maps.append(im)

    res = run_bass_kernel_spmd(nc, in_maps, list(range(NC)))
    full = np.concatenate([res.results[m]["out3"] for m in range(NC)])
    full = full.reshape(N, 3, F)
    return (np.ascontiguousarray(full[:, 0]),
            np.ascontiguousarray(full[:, 1]),
            np.ascontiguousarray(full[:, 2]))
